# revision 1
# baseline (speedup 1.0000x reference)
"""Causal multi-head attention (B=2, T=4096, C=768, H=12) on 8 Trainium2 cores.

Sharding: core c handles batch b=c//4 and heads 3*(c%4)..3*(c%4)+2 for the
QKV projections and flash attention; one 8-way AllToAll PER HEAD redistributes
that head's attention output so core j holds ALL heads for tq strip j (both
batches), then each core runs the Wo projection for its 2x512 output rows.
The per-head exchanges fire as each head finishes and hide under the next
head's attention; phase 3 consumes the head-major gathered layout against
host-permuted Wo rows (the contraction is order-invariant).

All matmuls run as float32r (tf32-class, full PE rate at free-dim >= 256).
Flash attention uses no-max-subtraction softmax (scores are O(+-5) here, exp
is safe in fp32) with the denominator computed by an appended ones-column on V
(output free-dim 65 = 64 dims + rowsum). Strips 0-3 run strips-outer so all
three heads' exp work fills ACT during the projection-heavy ramp; strips 4-7
run heads-outer so each AllToAll fires early.
"""
import numpy as np
from contextlib import ExitStack

import concourse.bass as bass
import concourse.mybir as mybir
import concourse.tile as tile
from concourse import bacc
from concourse.bass_utils import run_bass_kernel_spmd
from concourse.masks import make_identity, make_upper_triangular

T = 4096
C = 768
H = 12
D = 64
HPC = 3            # heads per core
MPC = HPC * D      # 192 projected dims per core
NCORES = 8
NTB = T // 128     # 32 tk blocks
NQB = T // 512     # 8 tq strips
CB = C // 128      # 6 contraction blocks
f32 = mybir.dt.float32
f32r = mybir.dt.float32r
EXP = mybir.ActivationFunctionType.Exp

_CACHE = {}


def _build():
    nc = bacc.Bacc(None, target_bir_lowering=False, num_devices=NCORES)
    x_in = nc.declare_dram_parameter("x", [T, C], f32r, isOutput=False)
    # weight params typed float32r: the PE rounds f32 operands to f32r
    # internally anyway, so binding raw f32 bits is value-preserving while
    # letting DMA feed matmuls directly (no on-chip rounding copies).
    wq_in = nc.declare_dram_parameter("wq", [C, MPC], f32r, isOutput=False)
    wk_in = nc.declare_dram_parameter("wk", [C, MPC], f32r, isOutput=False)
    wv_in = nc.declare_dram_parameter("wv", [C, MPC], f32r, isOutput=False)
    bq_in = nc.declare_dram_parameter("bq", [MPC], f32, isOutput=False)
    bk_in = nc.declare_dram_parameter("bk", [MPC], f32, isOutput=False)
    bv_in = nc.declare_dram_parameter("bv", [MPC], f32, isOutput=False)
    wo_in = nc.declare_dram_parameter("wo", [C, C], f32r, isOutput=False)
    bo_in = nc.declare_dram_parameter("bo", [C], f32, isOutput=False)
    out_d = nc.declare_dram_parameter("out", [2, 512, C], f32, isOutput=True)

    with tile.TileContext(nc) as tc, ExitStack() as ctx:
        singles = ctx.enter_context(tc.tile_pool(name="singles", bufs=1))
        dram = ctx.enter_context(tc.tile_pool(name="dram", bufs=1, space="DRAM"))

        # ---- static tiles -------------------------------------------------
        # identity in f32r: transpose-mode matmuls then run 1.5 cyc/row vs 2.0
        identity = singles.tile([128, 128], f32r)
        # mask[:, 0:128] = 0, mask[:, 128:256] = upper-tri (c >= r)
        mask = singles.tile([128, 256], f32)
        nc.gpsimd.memset(mask[:, 0:128], 0.0)
        make_upper_triangular(nc, mask[:, 128:256], val=1.0)

        # ---- weights -> SBUF (f32r params: straight DMA, no rounding copies)
        wq_r = singles.tile([128, CB, MPC], f32r)
        wk_r = singles.tile([128, CB, MPC], f32r)
        # wv padded to 256 free cols (zeros) so the v-proj matmul has N=256
        wv_r = singles.tile([128, CB, 256], f32r)
        wo_r = singles.tile([128, CB, C], f32r)
        # identity FIRST on the gpsimd queue so the first transpose isn't
        # gated behind the weight DMAs below
        with tc.tile_pool(name="idstage", bufs=1) as idstage:
            idf = idstage.tile([128, 128], f32)
            make_identity(nc, idf)
            nc.vector.tensor_copy(identity, idf)
        # weight loads ride SWDGE (gpsimd) so they don't queue ahead of the
        # first x-strip loads on the HWDGE (sync) queues
        nc.gpsimd.dma_start(out=wq_r, in_=wq_in.rearrange("(cb p) m -> p cb m", p=128))
        nc.gpsimd.dma_start(out=wk_r, in_=wk_in.rearrange("(cb p) m -> p cb m", p=128))
        nc.gpsimd.dma_start(
            out=wv_r[:, :, 0:MPC], in_=wv_in.rearrange("(cb p) m -> p cb m", p=128)
        )
        # combined q-tail/k-tail weight: one [128, 512] projection matmul set
        # yields q2 rows 0-63 and k2 rows 64-127
        wqk_t = singles.tile([128, CB, 128], f32r)
        nc.gpsimd.dma_start(
            out=wqk_t[:, :, 0:64],
            in_=wq_in.rearrange("(cb p) m -> p cb m", p=128)[:, :, 128:MPC],
        )
        nc.gpsimd.dma_start(
            out=wqk_t[:, :, 64:128],
            in_=wk_in.rearrange("(cb p) m -> p cb m", p=128)[:, :, 128:MPC],
        )
        with tc.tile_pool(name="wstage", bufs=1) as wstage:
            zpad = wstage.tile([128, CB, 64], f32)
            nc.vector.memset(zpad, 0.0)
            nc.vector.tensor_copy(wv_r[:, :, MPC:256], zpad)
            # pre-trigger the exp table load so its ~2.7us hides in the ramp
            warm = wstage.tile([1, 2], f32)
            nc.vector.memset(warm, 0.0)
            nc.scalar.activation(warm[:, 1:2], warm[:, 0:1], EXP, scale=1.0)

        # ---- biases -------------------------------------------------------
        bq_c = singles.tile([128, 1], f32)
        bk_c = singles.tile([128, 1], f32)
        bq_c2 = singles.tile([64, 1], f32)
        bk_c2h = singles.tile([128, 1], f32)  # k-tail bias parked at rows 64-127
        nc.gpsimd.dma_start(out=bq_c, in_=bq_in[0:128].unsqueeze(1))
        nc.gpsimd.dma_start(out=bk_c, in_=bk_in[0:128].unsqueeze(1))
        nc.gpsimd.dma_start(out=bq_c2, in_=bq_in[128:MPC].unsqueeze(1))
        nc.gpsimd.dma_start(out=bk_c2h[64:128, :], in_=bk_in[128:MPC].unsqueeze(1))
        bv_b = singles.tile([128, MPC], f32)
        nc.gpsimd.dma_start(
            out=bv_b,
            in_=bass.AP(tensor=bv_in.ap().tensor, offset=0, ap=[[0, 128]] + bv_in.ap().ap),
        )
        bo_b = singles.tile([128, C], f32)

        # ---- persistent activation buffers --------------------------------
        # qT/kT per head, d on partitions: heads 0,1 packed into [128, T]
        q01 = singles.tile([128, T], f32r)
        k01 = singles.tile([128, T], f32r)
        q2 = singles.tile([64, T], f32r)
        k2 = singles.tile([64, T], f32r)
        # V + ones column, per tk block and head: [128, 32, 3, 65]
        v1 = singles.tile([128, NTB, HPC, D + 1], f32r)
        ones_t = singles.tile([128, NTB, HPC], f32)
        nc.vector.memset(ones_t, 1.0)
        nc.vector.tensor_copy(v1[:, :, :, D], ones_t)

        # one tile pair per head: head h's AllToAll fires as soon as that
        # head's attention finishes, hiding under the next head's compute
        a2a_in = tuple(
            dram.tile([NCORES, D, 512], f32r, name=f"a2a_in{h}") for h in range(HPC)
        )
        a2a_out = tuple(
            dram.tile([NCORES, D, 512], f32r, name=f"a2a_out{h}") for h in range(HPC)
        )

        # ---- main loop ----------------------------------------------------
        # Strips 0-3 run strips-outer (all heads per strip) so exp work fills
        # ACT during the projection-heavy ramp; strips 4-7 run heads-outer so
        # each head's AllToAll fires early and hides under the next head's
        # attention (projections for strips 4-7 ride along head 0's pass).
        with (
            tc.tile_pool(name="pm", bufs=1) as pm,
            tc.tile_pool(name="psm", bufs=1, space="PSUM") as psm,
            tc.tile_pool(name="drm", bufs=1, space="DRAM") as drm,
        ):
            def do_proj(it):
                xT = pm.tile([128, CB, 512], f32r, tag="xT", bufs=2, name="xT")
                xns = []
                for hf in range(2):
                    xn = pm.tile([128, 2, C], f32r, tag="xn", bufs=3, name="xn")
                    nc.sync.dma_start(
                        out=xn,
                        in_=x_in[
                            512 * it + 256 * hf : 512 * it + 256 * (hf + 1), :
                        ].rearrange("(tb p) c -> p tb c", p=128),
                    )
                    xns.append(xn)
                for cb in range(CB):
                    ps_t = psm.tile([128, 512], f32r, tag="proj", bufs=2, name="ps_t")
                    for hf in range(2):
                        for tb in range(2):
                            nc.tensor.transpose(
                                ps_t[:, 256 * hf + 128 * tb : 256 * hf + 128 * (tb + 1)],
                                xns[hf][:, tb, 128 * cb : 128 * (cb + 1)],
                                identity,
                            )
                    nc.vector.tensor_copy(xT[:, cb, :], ps_t)
                for w_r, bc, dA in ((wq_r, bq_c, q01), (wk_r, bk_c, k01)):
                    psA = psm.tile([128, 512], f32, tag="proj", bufs=2, name="psA")
                    for cb in range(CB):
                        nc.tensor.matmul(
                            psA, w_r[:, cb, 0:128], xT[:, cb, :],
                            start=(cb == 0), stop=(cb == CB - 1),
                        )
                    nc.vector.tensor_scalar_add(
                        dA[:, 512 * it : 512 * (it + 1)], psA, bc
                    )
                # q-tail (head 2 q, rows 0-63) + k-tail (head 2 k, rows 64-127)
                # in one accumulation; k half realigned to base 0 via DMA
                psB = psm.tile([128, 512], f32, tag="proj", bufs=2, name="psB")
                for cb in range(CB):
                    nc.tensor.matmul(
                        psB, wqk_t[:, cb, :], xT[:, cb, :],
                        start=(cb == 0), stop=(cb == CB - 1),
                    )
                nc.vector.tensor_scalar_add(
                    q2[:, 512 * it : 512 * (it + 1)], psB[0:64, :], bq_c2
                )
                ktmp = pm.tile([128, 512], f32r, tag="ktmp", bufs=2, name="ktmp")
                nc.vector.tensor_scalar_add(
                    ktmp[64:128, :], psB[64:128, :], bk_c2h[64:128, :]
                )
                nc.sync.dma_start(
                    out=k2[:, 512 * it : 512 * (it + 1)], in_=ktmp[64:128, :]
                )
                for tb in range(4):
                    psV = psm.tile([128, 256], f32, tag="proj", bufs=2, name="psV")
                    for cb in range(CB):
                        nc.tensor.matmul(
                            psV, xT[:, cb, 128 * tb : 128 * (tb + 1)], wv_r[:, cb, :],
                            start=(cb == 0), stop=(cb == CB - 1),
                        )
                    tk = 4 * it + tb
                    nc.vector.tensor_add(
                        v1[:, tk, :, 0:D],
                        psV[:, 0:MPC].rearrange("p (h d) -> p h d", h=HPC),
                        bv_b.rearrange("p (h d) -> p h d", h=HPC),
                    )

            def do_attn(h, iq):
                qh = (q01[0:64], q01[64:128], q2[0:64])[h]
                kh = (k01[0:64], k01[64:128], k2[0:64])[h]
                ps_o = psm.tile([65, 512], f32, tag="o", bufs=2, name="ps_o")
                qs = qh[:, 512 * iq : 512 * (iq + 1)]
                # full tk blocks in pairs: one [128, 1024] exp, no masking
                for p in range(2 * iq):
                    ik0, ik1 = 2 * p, 2 * p + 1
                    ps2 = psm.tile([128, 1024], f32, tag="s", bufs=2, name="ps2")
                    nc.tensor.matmul(
                        ps2[:, 0:512], kh[:, 128 * ik0 : 128 * (ik0 + 1)], qs,
                        start=True, stop=True,
                    )
                    nc.tensor.matmul(
                        ps2[:, 512:1024], kh[:, 128 * ik1 : 128 * (ik1 + 1)], qs,
                        start=True, stop=True,
                    )
                    pT = pm.tile([128, 1024], f32r, tag="pT", bufs=3, name="pT")
                    nc.scalar.activation(pT, ps2, EXP, scale=0.125)
                    nc.tensor.matmul(
                        ps_o, v1[:, ik0, h, :], pT[:, 0:512],
                        start=(ik0 == 0), stop=False,
                    )
                    nc.tensor.matmul(
                        ps_o, v1[:, ik1, h, :], pT[:, 512:1024],
                        start=False, stop=False,
                    )
                # diagonal region: 4 single blocks with causal masking
                for j in range(4):
                    ik = 4 * iq + j
                    col0 = 0 if j < 1 else (128 if j == 1 else 256)
                    ps2 = psm.tile([128, 1024], f32, tag="s", bufs=2, name="ps2")
                    nc.tensor.matmul(
                        ps2[:, col0:512],
                        kh[:, 128 * ik : 128 * (ik + 1)],
                        qh[:, 512 * iq + col0 : 512 * (iq + 1)],
                        start=True, stop=True,
                    )
                    pT = pm.tile([128, 1024], f32r, tag="pT", bufs=3, name="pT")
                    nc.scalar.activation(pT[:, col0:512], ps2[:, col0:512], EXP, scale=0.125)
                    if j == 3:
                        nc.vector.tensor_mul(pT[:, 256:512], pT[:, 256:512], mask)
                    else:
                        nc.vector.tensor_mul(
                            pT[:, col0 : col0 + 128],
                            pT[:, col0 : col0 + 128],
                            mask[:, 128:256],
                        )
                    nc.tensor.matmul(
                        ps_o[:, col0:], v1[:, ik, h, :], pT[:, col0:512],
                        start=(ik == 0), stop=(j == 3),
                    )
                recip = pm.tile([128, 512], f32, tag="rc", bufs=3, name="recip")
                nc.vector.reciprocal(recip[64:65, :], ps_o[64:65, :])
                rc_d = drm.tile([512], f32, tag="rcd", bufs=3, name="rc_d")
                nc.sync.dma_start(out=rc_d.unsqueeze(0), in_=recip[64:65, :])
                bcast = pm.tile([64, 512], f32, tag="bc", bufs=3, name="bcast")
                nc.sync.dma_start(
                    out=bcast,
                    in_=bass.AP(tensor=rc_d.tensor, offset=rc_d[:].offset, ap=[[0, 64]] + rc_d[:].ap),
                )
                att_n = pm.tile([64, 512], f32r, tag="an", bufs=3, name="att_n")
                nc.vector.tensor_mul(att_n, ps_o[0:64, :], bcast)
                nc.sync.dma_start(out=a2a_in[h][iq, :, :], in_=att_n)

            for iq in range(4):
                do_proj(iq)
                for h in range(HPC):
                    do_attn(h, iq)
            for h in range(HPC):
                for iq in range(4, NQB):
                    if h == 0:
                        do_proj(iq)
                    do_attn(h, iq)
                nc.gpsimd.collective_compute(
                    "AllToAll",
                    mybir.AluOpType.bypass,
                    replica_groups=[list(range(NCORES))],
                    ins=[a2a_in[h][:]],
                    outs=[a2a_out[h][:]],
                )

        # ---- phase 3: output projection -----------------------------------
        # gathered layout is head-major: flats[h] rows = 64*src + d; the host
        # permutes Wo's rows to match (see kernel()).
        flats = tuple(a.rearrange("s d t -> (s d) t") for a in a2a_out)  # [512, 512]
        with (
            tc.tile_pool(name="p3", bufs=1) as p3,
            tc.tile_pool(name="ps3", bufs=1, space="PSUM") as ps3,
        ):
            nc.sync.dma_start(out=wo_r, in_=wo_in.rearrange("(cb p) m -> p cb m", p=128))
            nc.sync.dma_start(
                out=bo_b,
                in_=bass.AP(tensor=bo_in.ap().tensor, offset=0, ap=[[0, 128]] + bo_in.ap().ap),
            )
            for bb in range(2):
                for tb in range(4):
                    ps_a = ps3.tile([128, 512], f32, tag="a", bufs=4)
                    ps_b = ps3.tile([128, 256], f32, tag="b", bufs=4)
                    for idx in range(CB):
                        h_l, half = divmod(idx, 2)
                        lt = p3.tile([128, 128], f32r, tag="ltr", bufs=12)
                        nc.sync.dma_start(
                            out=lt,
                            in_=flats[h_l][
                                256 * bb + 128 * half : 256 * bb + 128 * (half + 1),
                                128 * tb : 128 * (tb + 1),
                            ],
                        )
                        nc.tensor.matmul(
                            ps_a, lt, wo_r[:, idx, 0:512],
                            start=(idx == 0), stop=(idx == CB - 1),
                        )
                        nc.tensor.matmul(
                            ps_b, lt, wo_r[:, idx, 512:C],
                            start=(idx == 0), stop=(idx == CB - 1),
                        )
                    out_t = p3.tile([128, C], f32, tag="ot", bufs=3)
                    nc.vector.tensor_add(out_t[:, 0:512], ps_a, bo_b[:, 0:512])
                    nc.vector.tensor_add(out_t[:, 512:C], ps_b, bo_b[:, 512:C])
                    nc.sync.dma_start(
                        out=out_d[bb, 128 * tb : 128 * (tb + 1), :], in_=out_t
                    )

    nc.finalize()
    return nc


def kernel(x, Wq, bq, Wk, bk, Wv, bv, Wo, bo):
    if "nc" not in _CACHE:
        _CACHE["nc"] = _build()
    nc = _CACHE["nc"]

    x = np.asarray(x, dtype=np.float32)
    # permute Wo rows from global head-dim order (192g + 64h + d) to the
    # head-major gathered layout (256h + 64g + d) used by phase 3
    perm = np.empty(C, dtype=np.int64)
    for h_l in range(HPC):
        for g in range(4):
            perm[256 * h_l + 64 * g : 256 * h_l + 64 * g + 64] = np.arange(
                MPC * g + D * h_l, MPC * g + D * h_l + D
            )
    wo_send = np.ascontiguousarray(np.asarray(Wo, np.float32)[perm, :])
    in_maps = []
    for c in range(NCORES):
        b, g = c // 4, c % 4
        sl = slice(MPC * g, MPC * (g + 1))
        in_maps.append({
            "x": np.ascontiguousarray(x[b]),
            "wq": np.ascontiguousarray(np.asarray(Wq, np.float32)[:, sl]),
            "wk": np.ascontiguousarray(np.asarray(Wk, np.float32)[:, sl]),
            "wv": np.ascontiguousarray(np.asarray(Wv, np.float32)[:, sl]),
            "bq": np.ascontiguousarray(np.asarray(bq, np.float32)[sl]),
            "bk": np.ascontiguousarray(np.asarray(bk, np.float32)[sl]),
            "bv": np.ascontiguousarray(np.asarray(bv, np.float32)[sl]),
            "wo": wo_send,
            "bo": np.ascontiguousarray(np.asarray(bo, np.float32)),
        })

    res = run_bass_kernel_spmd(nc, in_maps, core_ids=list(range(NCORES)))
    out = np.empty((2, T, C), dtype=np.float32)
    for j in range(NCORES):
        r = res.results[j]["out"]
        out[0, 512 * j : 512 * (j + 1), :] = r[0]
        out[1, 512 * j : 512 * (j + 1), :] = r[1]
    return out



# revision 35
# speedup vs baseline: 1.2748x; 1.2748x over previous
"""Causal multi-head attention (B=2, T=4096, C=768, H=12) on 8 Trainium2 cores.

Sharding: core c handles batch b=c//4 and heads 3*(c%4)..3*(c%4)+2 for the
QKV projections and flash attention; one 8-way AllToAll PER HEAD redistributes
that head's attention output (bf16) so core j holds ALL heads for tq strip j
(both batches); each core then runs the Wo projection for its 2x512 output
rows, one per-head chunk at a time as each head's AllToAll lands.

Key optimizations vs the f32r baseline:
- x arrives HOST-PRETRANSPOSED as [C, T], so the per-strip PE transposes and
  PSUM->SBUF copies are gone; projections read xT strips straight from DRAM.
- Scores matmuls run in fp8e4m3 with DoubleRow perf mode (0.5 cyc/row): q/k
  are stored as [32, 2, T] fp8 tiles (contraction dim d=2p+i folded onto 32
  partitions x 2 column blocks) via one staging fp8 bias-add + one fold DMA.
- Everything downstream of exp is bf16 (pT, V, attention outputs), halving
  the per-head AllToAll to 512KB.
- Head-rotated schedule: head 2 finishes its strips first and its AllToAll
  fires ~90us before the end; phase-3 Wo chunks are interleaved per head so
  only the last head's 28us collective + 5us Wo chunk trail the main loop.
Flash attention uses no-max-subtraction softmax (scores are O(+-5), exp is
safe) with the denominator computed by an appended ones-column on V.
"""
import numpy as np
from contextlib import ExitStack

import concourse.bass as bass
import concourse.mybir as mybir
import concourse.tile as tile
from concourse import bacc
from concourse.bass_utils import run_bass_kernel_spmd
from concourse.masks import make_upper_triangular

T = 4096
C = 768
H = 12
D = 64
HPC = 3            # heads per core
MPC = HPC * D      # 192 projected dims per core
NCORES = 8
NTB = T // 128     # 32 tk blocks
NQB = T // 512     # 8 tq strips
CB = C // 128      # 6 contraction blocks
f32 = mybir.dt.float32
f32r = mybir.dt.float32r
bf16 = mybir.dt.bfloat16
f8 = mybir.dt.float8e4
EXP = mybir.ActivationFunctionType.Exp
DR = mybir.MatmulPerfMode.DoubleRow
i32 = mybir.dt.int32

# Schraudolph exp-approximation on DVE for a fraction of the score blocks:
# exp(s/8) ~= bitcast_f32(int32(s*EXP_A + EXP_B)). Offloads the ACT engine
# (the critical engine) onto idle DVE cycles. ~1.8% RMS per-weight error on
# the routed blocks; numerator and denominator use the same approximation so
# softmax row sums stay exact.
EXP_A = float(0.125 * (1 << 23) / np.log(2.0))
EXP_B = float(127 * (1 << 23) - 486408 + 0.5)
DVE_EXP_MOD = 7  # route pair p of (h, iq) to DVE when (p+iq+h) % MOD == 1

_CACHE = {}


def _build():
    nc = bacc.Bacc(None, target_bir_lowering=False, num_devices=NCORES)
    # x is pre-transposed on the host: [C, T] for this core's batch
    x_in = nc.declare_dram_parameter("x", [C, T], f32r, isOutput=False)
    wq_in = nc.declare_dram_parameter("wq", [C, MPC], f32r, isOutput=False)
    wk_in = nc.declare_dram_parameter("wk", [C, MPC], f32r, isOutput=False)
    wv_in = nc.declare_dram_parameter("wv", [C, MPC], f32r, isOutput=False)
    bq_in = nc.declare_dram_parameter("bq", [MPC], f32, isOutput=False)
    bk_in = nc.declare_dram_parameter("bk", [MPC], f32, isOutput=False)
    bv_in = nc.declare_dram_parameter("bv", [MPC], f32, isOutput=False)
    wo_in = nc.declare_dram_parameter("wo", [C, C], f32r, isOutput=False)
    bo_in = nc.declare_dram_parameter("bo", [C], f32, isOutput=False)
    out_d = nc.declare_dram_parameter("out", [2, 512, C], f32, isOutput=True)

    with tile.TileContext(nc) as tc, ExitStack() as ctx:
        singles = ctx.enter_context(tc.tile_pool(name="singles", bufs=1))
        dram = ctx.enter_context(tc.tile_pool(name="dram", bufs=1, space="DRAM"))

        # ---- static tiles -------------------------------------------------
        # mask[:, 0:128] = 0, mask[:, 128:256] = upper-tri (c >= r), bf16
        mask = singles.tile([128, 256], bf16)
        nc.gpsimd.memset(mask[:, 0:128], 0.0)
        make_upper_triangular(nc, mask[:, 128:256], val=1.0)

        # ---- weights -> SBUF (f32r params: straight DMA, no rounding copies)
        wq_r = singles.tile([128, CB, MPC], f32r)
        wk_r = singles.tile([128, CB, MPC], f32r)
        # wv padded to 256 free cols (zeros) so the v-proj matmul has N=256
        wv_r = singles.tile([128, CB, 256], f32r)
        wo_r = singles.tile([128, CB, C], f32r)
        # load order matters: attn(2, 0) is the first attention work and needs
        # wqk_t (its q/k come from the psB tail matmuls), so wqk_t loads
        # before wq/wk; wo's 2.4MB load is deferred into the main loop
        # combined q-tail/k-tail weight: one [128, 512] projection matmul set
        # yields q2 rows 0-63 and k2 rows 64-127
        wqk_t = singles.tile([128, CB, 128], f32r)
        nc.gpsimd.dma_start(
            out=wqk_t[:, :, 0:64],
            in_=wq_in.rearrange("(cb p) m -> p cb m", p=128)[:, :, 128:MPC],
        )
        nc.gpsimd.dma_start(
            out=wqk_t[:, :, 64:128],
            in_=wk_in.rearrange("(cb p) m -> p cb m", p=128)[:, :, 128:MPC],
        )
        nc.gpsimd.dma_start(out=wq_r, in_=wq_in.rearrange("(cb p) m -> p cb m", p=128))
        nc.gpsimd.dma_start(out=wk_r, in_=wk_in.rearrange("(cb p) m -> p cb m", p=128))
        nc.gpsimd.dma_start(
            out=wv_r[:, :, 0:MPC], in_=wv_in.rearrange("(cb p) m -> p cb m", p=128)
        )
        with tc.tile_pool(name="wstage", bufs=1) as wstage:
            zpad = wstage.tile([128, CB, 64], f32)
            nc.vector.memset(zpad, 0.0)
            nc.vector.tensor_copy(wv_r[:, :, MPC:256], zpad)
            # pre-trigger the exp table load so its ~2.7us hides in the ramp
            warm = wstage.tile([1, 2], f32)
            nc.vector.memset(warm, 0.0)
            nc.scalar.activation(warm[:, 1:2], warm[:, 0:1], EXP, scale=1.0)

        # ---- biases (tiny loads, issued on sync so they beat the weights) --
        bq_c = singles.tile([128, 1], f32)
        bk_c = singles.tile([128, 1], f32)
        # combined tail bias: rows 0-63 = bq[128:192], rows 64-127 = bk[128:192]
        bqk2 = singles.tile([128, 1], f32)
        nc.sync.dma_start(out=bqk2[0:64, :], in_=bq_in[128:MPC].unsqueeze(1))
        nc.sync.dma_start(out=bqk2[64:128, :], in_=bk_in[128:MPC].unsqueeze(1))
        nc.sync.dma_start(out=bq_c, in_=bq_in[0:128].unsqueeze(1))
        nc.sync.dma_start(out=bk_c, in_=bk_in[0:128].unsqueeze(1))
        bv_b = singles.tile([128, MPC], f32)
        nc.gpsimd.dma_start(
            out=bv_b,
            in_=bass.AP(tensor=bv_in.ap().tensor, offset=0, ap=[[0, 128]] + bv_in.ap().ap),
        )
        bo_b = singles.tile([128, C], f32)
        nc.gpsimd.dma_start(
            out=bo_b,
            in_=bass.AP(tensor=bo_in.ap().tensor, offset=0, ap=[[0, 128]] + bo_in.ap().ap),
        )

        # ---- persistent activation buffers --------------------------------
        # q/k per head in fp8, folded for DoubleRow: [32, 2, T], d = 2p+i
        q_f8 = tuple(singles.tile([32, 2, T], f8, name=f"qf8_{h}") for h in range(HPC))
        k_f8 = tuple(singles.tile([32, 2, T], f8, name=f"kf8_{h}") for h in range(HPC))
        # V + ones column, per tk block and head: [128, 32, 3, 65] bf16
        v1 = singles.tile([128, NTB, HPC, D + 1], bf16)
        ones_t = singles.tile([128, NTB, HPC], bf16)
        nc.vector.memset(ones_t, 1.0)
        nc.vector.tensor_copy(v1[:, :, :, D], ones_t)
        # phase-3 SBUF accumulator: [128 tokens, bb, tb, C] f32
        acc = singles.tile([128, 2, 4, C], f32)

        # one tile pair per head: head h's AllToAll fires as soon as that
        # head's attention finishes. Row 64 carries the UNNORMALIZED softmax
        # denominator; normalization happens receiver-side in phase 3, so the
        # per-strip critical chain is one bf16 copy + one DMA.
        a2a_in = tuple(
            dram.tile([NCORES, D + 1, 512], bf16, name=f"a2a_in{h}") for h in range(HPC)
        )
        a2a_out = tuple(
            dram.tile([NCORES, D + 1, 512], bf16, name=f"a2a_out{h}") for h in range(HPC)
        )

        # ---- main loop ----------------------------------------------------
        with (
            tc.tile_pool(name="pm", bufs=1) as pm,
            tc.tile_pool(name="psm", bufs=1, space="PSUM") as psm,
            tc.tile_pool(name="drm", bufs=1, space="DRAM") as drm,
        ):
            psm_proj_cm = tc.tile_pool(name="psmp", bufs=1, space="PSUM")
            psm_proj = psm_proj_cm.__enter__()

            def do_proj(it):
                xT = pm.tile([128, CB, 512], f32r, tag="xT", bufs=2, name="xT")
                src = x_in.rearrange("(cb p) t -> p cb t", p=128)
                # split by cb so the first accumulation matmuls start after
                # only half the strip's x has landed
                for hf in range(2):
                    nc.sync.dma_start(
                        out=xT[:, 3 * hf : 3 * (hf + 1), :],
                        in_=src[:, 3 * hf : 3 * (hf + 1), 512 * it : 512 * (it + 1)],
                    )
                # head-2 q/k tail FIRST (attn(2, iq) runs right after this
                # projection): q2 rows 0-63, k2 rows 64-127 in one matmul set
                psB = psm_proj.tile([128, 512], f32, tag="proj", bufs=2, name="psB")
                for cb in range(CB):
                    nc.tensor.matmul(
                        psB, wqk_t[:, cb, :], xT[:, cb, :],
                        start=(cb == 0), stop=(cb == CB - 1),
                    )
                stgB = pm.tile([128, 512], f8, tag="stg", bufs=3, name="stgB")
                nc.vector.tensor_scalar_add(stgB, psB, bqk2)
                nc.gpsimd.dma_start(
                    out=q_f8[2][:, :, 512 * it : 512 * (it + 1)], in_=stgB[0:64, :]
                )
                nc.gpsimd.dma_start(
                    out=k_f8[2][:, :, 512 * it : 512 * (it + 1)], in_=stgB[64:128, :]
                )
                for w_r, bc, dsts in (
                    (wq_r, bq_c, (q_f8[0], q_f8[1])),
                    (wk_r, bk_c, (k_f8[0], k_f8[1])),
                ):
                    psA = psm_proj.tile([128, 512], f32, tag="proj", bufs=2, name="psA")
                    for cb in range(CB):
                        nc.tensor.matmul(
                            psA, w_r[:, cb, 0:128], xT[:, cb, :],
                            start=(cb == 0), stop=(cb == CB - 1),
                        )
                    stg = pm.tile([128, 512], f8, tag="stg", bufs=3, name="stg")
                    nc.vector.tensor_scalar_add(stg, psA, bc)
                    for hh in range(2):
                        nc.gpsimd.dma_start(
                            out=dsts[hh][:, :, 512 * it : 512 * (it + 1)],
                            in_=stg[64 * hh : 64 * (hh + 1), :],
                        )
                for tb in range(4):
                    psV = psm_proj.tile([128, 256], f32, tag="proj", bufs=2, name="psV")
                    for cb in range(CB):
                        nc.tensor.matmul(
                            psV, xT[:, cb, 128 * tb : 128 * (tb + 1)], wv_r[:, cb, :],
                            start=(cb == 0), stop=(cb == CB - 1),
                        )
                    tk = 4 * it + tb
                    nc.vector.tensor_add(
                        v1[:, tk, :, 0:D],
                        psV[:, 0:MPC].rearrange("p (h d) -> p h d", h=HPC),
                        bv_b.rearrange("p (h d) -> p h d", h=HPC),
                    )

            def do_attn(h, iq):
                qh, kh = q_f8[h], k_f8[h]
                ps_o = psm.tile([65, 512], f32, tag="o", bufs=2, name="ps_o")
                qs = qh[:, :, 512 * iq : 512 * (iq + 1)]
                # full tk blocks in pairs: one [128, 1024] exp, no masking
                for p in range(2 * iq):
                    ik0, ik1 = 2 * p, 2 * p + 1
                    ps2 = psm.tile([128, 1024], f32, tag="s", bufs=2, name="ps2")
                    nc.tensor.matmul(
                        ps2[:, 0:512], kh[:, :, 128 * ik0 : 128 * (ik0 + 1)], qs,
                        start=True, stop=True, perf_mode=DR,
                    )
                    nc.tensor.matmul(
                        ps2[:, 512:1024], kh[:, :, 128 * ik1 : 128 * (ik1 + 1)], qs,
                        start=True, stop=True, perf_mode=DR,
                    )
                    pT = pm.tile([128, 1024], bf16, tag="pT", bufs=3, name="pT")
                    if DVE_EXP_MOD and (p + iq + h) % DVE_EXP_MOD == 1:
                        yi = pm.tile([128, 1024], i32, tag="yi", bufs=2, name="yi")
                        nc.vector.tensor_scalar(
                            yi, ps2, EXP_A, EXP_B,
                            mybir.AluOpType.mult, mybir.AluOpType.add,
                        )
                        nc.vector.tensor_copy(pT, yi[:, :].bitcast(f32))
                    else:
                        nc.scalar.activation(pT, ps2, EXP, scale=0.125)
                    nc.tensor.matmul(
                        ps_o, v1[:, ik0, h, :], pT[:, 0:512],
                        start=(ik0 == 0), stop=False,
                    )
                    nc.tensor.matmul(
                        ps_o, v1[:, ik1, h, :], pT[:, 512:1024],
                        start=False, stop=False,
                    )
                # diagonal region: 4 causal blocks, TWO exps via paired tiles
                # (j0 full + j1 full share one [128,1024] exp; j2+j3 share a
                # strided-AP exp over their two 256-col regions)
                ik0 = 4 * iq
                qs_c = qh[:, :, 512 * iq : 512 * (iq + 1)]
                ps2a = psm.tile([128, 1024], f32, tag="s", bufs=2, name="ps2")
                nc.tensor.matmul(
                    ps2a[:, 0:512], kh[:, :, 128 * ik0 : 128 * (ik0 + 1)],
                    qs_c, start=True, stop=True, perf_mode=DR,
                )
                nc.tensor.matmul(
                    ps2a[:, 512:1024], kh[:, :, 128 * (ik0 + 1) : 128 * (ik0 + 2)],
                    qs_c, start=True, stop=True, perf_mode=DR,
                )
                pTa = pm.tile([128, 1024], bf16, tag="pT", bufs=3, name="pT")
                nc.scalar.activation(pTa, ps2a, EXP, scale=0.125)
                nc.vector.tensor_mul(pTa[:, 0:128], pTa[:, 0:128], mask[:, 128:256])
                nc.vector.tensor_mul(pTa[:, 512:768], pTa[:, 512:768], mask)
                nc.tensor.matmul(
                    ps_o, v1[:, ik0, h, :], pTa[:, 0:512],
                    start=(ik0 == 0), stop=False,
                )
                nc.tensor.matmul(
                    ps_o[:, 128:], v1[:, ik0 + 1, h, :], pTa[:, 640:1024],
                    start=False, stop=False,
                )
                ps2b = psm.tile([128, 1024], f32, tag="s", bufs=2, name="ps2")
                qs_d = qh[:, :, 512 * iq + 256 : 512 * (iq + 1)]
                nc.tensor.matmul(
                    ps2b[:, 256:512], kh[:, :, 128 * (ik0 + 2) : 128 * (ik0 + 3)],
                    qs_d, start=True, stop=True, perf_mode=DR,
                )
                nc.tensor.matmul(
                    ps2b[:, 768:1024], kh[:, :, 128 * (ik0 + 3) : 128 * (ik0 + 4)],
                    qs_d, start=True, stop=True, perf_mode=DR,
                )
                pTb = pm.tile([128, 1024], bf16, tag="pT", bufs=3, name="pT")
                pTb_v = bass.AP(
                    tensor=pTb.tensor, offset=pTb[:, 256:512].offset,
                    ap=[list(pTb[:, 256:512].ap[0]), [512, 2], [1, 256]],
                )
                ps2b_v = bass.AP(
                    tensor=ps2b.tensor, offset=ps2b[:, 256:512].offset,
                    ap=[list(ps2b[:, 256:512].ap[0]), [512, 2], [1, 256]],
                )
                nc.scalar.activation(pTb_v, ps2b_v, EXP, scale=0.125)
                nc.vector.tensor_mul(pTb[:, 256:384], pTb[:, 256:384], mask[:, 128:256])
                nc.vector.tensor_mul(pTb[:, 768:1024], pTb[:, 768:1024], mask)
                nc.tensor.matmul(
                    ps_o[:, 256:], v1[:, ik0 + 2, h, :], pTb[:, 256:512],
                    start=False, stop=False,
                )
                nc.tensor.matmul(
                    ps_o[:, 256:], v1[:, ik0 + 3, h, :], pTb[:, 768:1024],
                    start=False, stop=True,
                )
                att_c = pm.tile([65, 512], bf16, tag="an", bufs=3, name="att_c")
                nc.vector.tensor_copy(att_c, ps_o)
                nc.sync.dma_start(out=a2a_in[h][iq, :, :], in_=att_c)
                return att_c

            def do_a2a(h):
                nc.gpsimd.collective_compute(
                    "AllToAll",
                    mybir.AluOpType.bypass,
                    replica_groups=[list(range(NCORES))],
                    ins=[a2a_in[h][:]],
                    outs=[a2a_out[h][:]],
                )

            def phase3(h, stage, psm_p3, gate=None):
                # gathered rows for head h: flats rows 64*src + d; chunk
                # (bb, half) = rows 256bb+128half.. = a2a_out[h][4bb+2half:+2]
                ltbs = []
                for bb in range(2):
                    for half in range(2):
                        s0 = 4 * bb + 2 * half
                        ltb = pm.tile([128, 512], bf16, tag="ltb", bufs=4, name="ltb")
                        if gate is not None:
                            # WAW token: the scheduling pass underestimates
                            # collective latency and would otherwise slot this
                            # chunk's matmuls ahead of remaining attention,
                            # head-of-line-blocking the PE queue on the
                            # collective. Tying the ltb tile to a late
                            # attention tile forces the chunk after that work.
                            nc.vector.tensor_copy(
                                ltb[64:65, 0:1], gate[64:65, 0:1]
                            )
                        nc.gpsimd.dma_start(
                            out=ltb, in_=a2a_out[h][s0 : s0 + 2, 0:D, :]
                        )
                        # denominators: row 64 of each src, partition-broadcast
                        # from DRAM (64 copies each) into [128, 512]
                        den = pm.tile([128, 512], bf16, tag="den", bufs=4, name="den")
                        dsrc = a2a_out[h][s0 : s0 + 2, D : D + 1, :]
                        nc.gpsimd.dma_start(
                            out=den,
                            in_=bass.AP(
                                tensor=dsrc.tensor,
                                offset=dsrc.offset,
                                ap=[list(dsrc.ap[0]), [0, D], [1, 512]],
                            ),
                        )
                        # DVE has no divide ISA op: reciprocal then multiply
                        dre = pm.tile([128, 512], f32, tag="dre", bufs=4, name="dre")
                        nc.vector.reciprocal(dre, den)
                        ltn = pm.tile([128, 512], f32r, tag="ltn", bufs=4, name="ltn")
                        with nc.allow_low_precision(reason="f32r stores f32 bits"):
                            nc.vector.tensor_mul(ltn, ltb, dre)
                        ltbs.append(ltn)
                for bb in range(2):
                    for tb in range(4):
                        # one [128, 768] PSUM tile: cols 0-511 in bank A,
                        # 512-767 in bank B, separate accumulation groups
                        ps = psm_p3.tile([128, C], f32, tag="pa", bufs=1, name="pa")
                        for half in range(2):
                            lt = ltbs[2 * bb + half][:, 128 * tb : 128 * (tb + 1)]
                            nc.tensor.matmul(
                                ps[:, 0:512], lt, wo_r[:, 2 * h + half, 0:512],
                                start=(half == 0), stop=(half == 1),
                            )
                            nc.tensor.matmul(
                                ps[:, 512:C], lt, wo_r[:, 2 * h + half, 512:C],
                                start=(half == 0), stop=(half == 1),
                            )
                        a = acc[:, bb, tb, :]
                        if stage == 0:
                            nc.vector.tensor_add(a, ps, bo_b)
                        elif stage == 1:
                            nc.vector.tensor_add(a, a, ps)
                        else:
                            # final stage is the critical tail: out DMAs
                            # alternate between two queues
                            out_t = pm.tile([128, C], f32, tag="ot", bufs=3, name="out_t")
                            nc.vector.tensor_add(out_t, a, ps)
                            eng = nc.sync if (bb * 4 + tb) % 2 == 0 else nc.scalar
                            eng.dma_start(
                                out=out_d[bb, 128 * tb : 128 * (tb + 1), :], in_=out_t
                            )

            # strips 0-3: proj + all heads (head 2 first so its tail is early)
            for iq in range(4):
                do_proj(iq)
                for h in (2, 1, 0):
                    do_attn(h, iq)
            # strips 4-7: head 2 rides along the remaining projections; wo's
            # big load slips in here (gpsimd queue is past the strip-0 folds)
            for iq in range(4, NQB):
                do_proj(iq)
                do_attn(2, iq)
                if iq in (4, 5):
                    half = iq - 4
                    nc.gpsimd.dma_start(
                        out=wo_r[:, 3 * half : 3 * (half + 1), :],
                        in_=wo_in.rearrange("(cb p) m -> p cb m", p=128)[
                            :, 3 * half : 3 * (half + 1), :
                        ],
                    )
            psm_proj_cm.__exit__(None, None, None)
            psm_p3_cm = tc.tile_pool(name="psm3", bufs=1, space="PSUM")
            psm_p3 = psm_p3_cm.__enter__()
            do_a2a(2)
            for iq in range(4, NQB):
                g1 = do_attn(1, iq)
                if iq == 6:
                    gate_b = g1
            do_a2a(1)
            do_attn(0, 4)
            phase3(2, 0, psm_p3, gate=gate_b)
            do_attn(0, 5)
            g0 = do_attn(0, 6)
            do_attn(0, 7)
            do_a2a(0)
            phase3(1, 1, psm_p3, gate=g0)
            phase3(0, 2, psm_p3)
            psm_p3_cm.__exit__(None, None, None)

    nc.finalize()
    return nc


def kernel(x, Wq, bq, Wk, bk, Wv, bv, Wo, bo):
    if "nc" not in _CACHE:
        _CACHE["nc"] = _build()
    nc = _CACHE["nc"]

    x = np.asarray(x, dtype=np.float32)
    # permute Wo rows from global head-dim order (192g + 64h + d) to the
    # head-major gathered layout (256h + 64g + d) used by phase 3
    perm = np.empty(C, dtype=np.int64)
    for h_l in range(HPC):
        for g in range(4):
            perm[256 * h_l + 64 * g : 256 * h_l + 64 * g + 64] = np.arange(
                MPC * g + D * h_l, MPC * g + D * h_l + D
            )
    wo_send = np.ascontiguousarray(np.asarray(Wo, np.float32)[perm, :])
    in_maps = []
    for c in range(NCORES):
        b, g = c // 4, c % 4
        sl = slice(MPC * g, MPC * (g + 1))
        in_maps.append({
            "x": np.ascontiguousarray(x[b].T),
            "wq": np.ascontiguousarray(np.asarray(Wq, np.float32)[:, sl]),
            "wk": np.ascontiguousarray(np.asarray(Wk, np.float32)[:, sl]),
            "wv": np.ascontiguousarray(np.asarray(Wv, np.float32)[:, sl]),
            "bq": np.ascontiguousarray(np.asarray(bq, np.float32)[sl]),
            "bk": np.ascontiguousarray(np.asarray(bk, np.float32)[sl]),
            "bv": np.ascontiguousarray(np.asarray(bv, np.float32)[sl]),
            "wo": wo_send,
            "bo": np.ascontiguousarray(np.asarray(bo, np.float32)),
        })

    res = run_bass_kernel_spmd(nc, in_maps, core_ids=list(range(NCORES)))
    out = np.empty((2, T, C), dtype=np.float32)
    for j in range(NCORES):
        r = res.results[j]["out"]
        out[0, 512 * j : 512 * (j + 1), :] = r[0]
        out[1, 512 * j : 512 * (j + 1), :] = r[1]
    return out


# revision 54
# speedup vs baseline: 1.2900x; 1.0119x over previous
"""Causal multi-head attention (B=2, T=4096, C=768, H=12) on 8 Trainium2 cores.

Sharding: core c handles batch b=c//4 and heads 3*(c%4)..3*(c%4)+2 for the
QKV projections and flash attention; one 8-way AllToAll PER HEAD redistributes
that head's attention output (bf16) so core j holds ALL heads for tq strip j
(both batches); each core then runs the Wo projection for its 2x512 output
rows, one per-head chunk at a time as each head's AllToAll lands.

Key optimizations vs the f32r baseline:
- x arrives HOST-PRETRANSPOSED as [C, T], so the per-strip PE transposes and
  PSUM->SBUF copies are gone; projections read xT strips straight from DRAM.
- Scores matmuls run in fp8e4m3 with DoubleRow perf mode (0.5 cyc/row): q/k
  are stored as [32, 2, T] fp8 tiles (contraction dim d=2p+i folded onto 32
  partitions x 2 column blocks) via one staging fp8 bias-add + one fold DMA.
- Everything downstream of exp is bf16 (pT, V, attention outputs), halving
  the per-head AllToAll to 512KB.
- Head-rotated schedule: head 2 finishes its strips first and its AllToAll
  fires ~90us before the end; phase-3 Wo chunks are interleaved per head so
  only the last head's 28us collective + 5us Wo chunk trail the main loop.
Flash attention uses no-max-subtraction softmax (scores are O(+-5), exp is
safe) with the denominator computed by an appended ones-column on V.
"""
import numpy as np
from contextlib import ExitStack

import concourse.bass as bass
import concourse.mybir as mybir
import concourse.tile as tile
from concourse import bacc
from concourse.bass_utils import run_bass_kernel_spmd
from concourse.masks import make_identity, make_upper_triangular

T = 4096
C = 768
H = 12
D = 64
HPC = 3            # heads per core
MPC = HPC * D      # 192 projected dims per core
NCORES = 8
NTB = T // 128     # 32 tk blocks
NQB = T // 512     # 8 tq strips
CB = C // 128      # 6 contraction blocks
f32 = mybir.dt.float32
f32r = mybir.dt.float32r
bf16 = mybir.dt.bfloat16
f8 = mybir.dt.float8e4
EXP = mybir.ActivationFunctionType.Exp
DR = mybir.MatmulPerfMode.DoubleRow
i32 = mybir.dt.int32

# Schraudolph exp-approximation on DVE for a fraction of the score blocks:
# exp(s/8) ~= bitcast_f32(int32(s*EXP_A + EXP_B)). Offloads the ACT engine
# (the critical engine) onto idle DVE cycles. ~1.8% RMS per-weight error on
# the routed blocks; numerator and denominator use the same approximation so
# softmax row sums stay exact.
EXP_A = float(0.125 * (1 << 23) / np.log(2.0))
EXP_B = float(127 * (1 << 23) - 486408 + 0.5)
DVE_EXP_MOD = 0  # disabled: DVE detour stalls the in-order PV chain, and the
                 # ~2% weight noise would exhaust the 2e-2 error budget on top
                 # of fp8 scores (measured 1.82e-2 with fp8 alone)

_CACHE = {}


def _build():
    nc = bacc.Bacc(None, target_bir_lowering=False, num_devices=NCORES)
    # x is pre-transposed on the host: [C, T] for this core's batch
    x_in = nc.declare_dram_parameter("x", [C, T], f32r, isOutput=False)
    wq_in = nc.declare_dram_parameter("wq", [C, MPC], f32r, isOutput=False)
    wk_in = nc.declare_dram_parameter("wk", [C, MPC], f32r, isOutput=False)
    wv_in = nc.declare_dram_parameter("wv", [C, MPC], f32r, isOutput=False)
    bq_in = nc.declare_dram_parameter("bq", [MPC], f32, isOutput=False)
    bk_in = nc.declare_dram_parameter("bk", [MPC], f32, isOutput=False)
    bv_in = nc.declare_dram_parameter("bv", [MPC], f32, isOutput=False)
    wo_in = nc.declare_dram_parameter("wo", [C, C], f32r, isOutput=False)
    bo_in = nc.declare_dram_parameter("bo", [C], f32, isOutput=False)
    out_d = nc.declare_dram_parameter("out", [2, 512, C], f32, isOutput=True)

    with tile.TileContext(nc) as tc, ExitStack() as ctx:
        singles = ctx.enter_context(tc.tile_pool(name="singles", bufs=1))
        dram = ctx.enter_context(tc.tile_pool(name="dram", bufs=1, space="DRAM"))

        # ---- static tiles -------------------------------------------------
        # mask[:, 0:128] = 0, mask[:, 128:256] = upper-tri (c >= r), bf16
        mask = singles.tile([128, 256], bf16)
        nc.gpsimd.memset(mask[:, 0:128], 0.0)
        make_upper_triangular(nc, mask[:, 128:256], val=1.0)
        # identity for the phase-3 final accumulate (acc folded into PSUM by
        # an extra matmul so the tail needs no DVE adds)
        ident = singles.tile([128, 128], f32r)
        with tc.tile_pool(name="idstage", bufs=1) as idstage:
            idf = idstage.tile([128, 128], f32)
            make_identity(nc, idf)
            nc.vector.tensor_copy(ident, idf)

        # ---- weights -> SBUF (f32r params: straight DMA, no rounding copies)
        wq_r = singles.tile([128, CB, MPC], f32r)
        wk_r = singles.tile([128, CB, MPC], f32r)
        # wv padded to 256 free cols (zeros) so the v-proj matmul has N=256
        wv_r = singles.tile([128, CB, 256], f32r)
        wo_r = singles.tile([128, CB, C], f32r)
        # load order matters: attn(2, 0) is the first attention work and needs
        # wqk_t (its q/k come from the psB tail matmuls), so wqk_t loads
        # before wq/wk; wo's 2.4MB load is deferred into the main loop
        # combined q-tail/k-tail weight: one [128, 512] projection matmul set
        # yields q2 rows 0-63 and k2 rows 64-127
        wqk_t = singles.tile([128, CB, 128], f32r)
        nc.gpsimd.dma_start(
            out=wqk_t[:, :, 0:64],
            in_=wq_in.rearrange("(cb p) m -> p cb m", p=128)[:, :, 128:MPC],
        )
        nc.gpsimd.dma_start(
            out=wqk_t[:, :, 64:128],
            in_=wk_in.rearrange("(cb p) m -> p cb m", p=128)[:, :, 128:MPC],
        )
        nc.gpsimd.dma_start(out=wq_r, in_=wq_in.rearrange("(cb p) m -> p cb m", p=128))
        nc.gpsimd.dma_start(out=wk_r, in_=wk_in.rearrange("(cb p) m -> p cb m", p=128))
        # wv/bv/bo ride the sync queue inside the main loop so strip-0's q/k
        # fold DMAs aren't queued behind them on gpsimd (V work isn't needed
        # until the first PV matmul, well after the first scores+exp)
        with tc.tile_pool(name="wstage", bufs=1) as wstage:
            zpad = wstage.tile([128, CB, 64], f32)
            nc.vector.memset(zpad, 0.0)
            nc.vector.tensor_copy(wv_r[:, :, MPC:256], zpad)
            # pre-trigger the exp table load so its ~2.7us hides in the ramp
            warm = wstage.tile([1, 2], f32)
            nc.vector.memset(warm, 0.0)
            nc.scalar.activation(warm[:, 1:2], warm[:, 0:1], EXP, scale=1.0)

        # ---- biases: tiny loads on the idle ACT queue, so the sync queue
        # carries nothing but x strips during the ramp ----------------------
        bq_c = singles.tile([128, 1], f32)
        bk_c = singles.tile([128, 1], f32)
        # combined tail bias: rows 0-63 = bq[128:192], rows 64-127 = bk[128:192]
        bqk2 = singles.tile([128, 1], f32)
        nc.scalar.dma_start(out=bqk2[0:64, :], in_=bq_in[128:MPC].unsqueeze(1))
        nc.scalar.dma_start(out=bqk2[64:128, :], in_=bk_in[128:MPC].unsqueeze(1))
        nc.scalar.dma_start(out=bq_c, in_=bq_in[0:128].unsqueeze(1))
        nc.scalar.dma_start(out=bk_c, in_=bk_in[0:128].unsqueeze(1))
        bv_b = singles.tile([128, MPC], f32)
        bo_b = singles.tile([128, C], f32)

        def load_v_weights():
            # the ACT queue is idle during the ramp; using it keeps these off
            # the SP queue (x strips) and Pool queue (q/k folds)
            nc.scalar.dma_start(
                out=wv_r[:, :, 0:MPC], in_=wv_in.rearrange("(cb p) m -> p cb m", p=128)
            )
            nc.scalar.dma_start(
                out=bv_b,
                in_=bass.AP(tensor=bv_in.ap().tensor, offset=0, ap=[[0, 128]] + bv_in.ap().ap),
            )
            nc.scalar.dma_start(
                out=bo_b,
                in_=bass.AP(tensor=bo_in.ap().tensor, offset=0, ap=[[0, 128]] + bo_in.ap().ap),
            )

        # ---- persistent activation buffers --------------------------------
        # q/k per head in fp8, folded for DoubleRow: [32, 2, T], d = 2p+i
        q_f8 = tuple(singles.tile([32, 2, T], f8, name=f"qf8_{h}") for h in range(HPC))
        k_f8 = tuple(singles.tile([32, 2, T], f8, name=f"kf8_{h}") for h in range(HPC))
        # V + ones column, per tk block and head: [128, 32, 3, 65] bf16
        v1 = singles.tile([128, NTB, HPC, D + 1], bf16)
        ones_t = singles.tile([128, NTB, HPC], bf16)
        nc.vector.memset(ones_t, 1.0)
        nc.vector.tensor_copy(v1[:, :, :, D], ones_t)
        # phase-3 SBUF accumulator: [128 tokens, bb, tb, C]; f32r so the
        # final chunk can feed it back through the PE as a moving operand
        acc = singles.tile([128, 2, 4, C], f32r)

        # one tile pair per head: head h's AllToAll fires as soon as that
        # head's attention finishes. Row 64 carries the UNNORMALIZED softmax
        # denominator; normalization happens receiver-side in phase 3, so the
        # per-strip critical chain is one bf16 copy + one DMA.
        a2a_in = tuple(
            dram.tile([NCORES, D + 1, 512], bf16, name=f"a2a_in{h}") for h in range(HPC)
        )
        a2a_out = tuple(
            dram.tile([NCORES, D + 1, 512], bf16, name=f"a2a_out{h}") for h in range(HPC)
        )

        # ---- main loop ----------------------------------------------------
        with (
            tc.tile_pool(name="pm", bufs=1) as pm,
            tc.tile_pool(name="drm", bufs=1, space="DRAM") as drm,
        ):
            psm_att_cm = tc.tile_pool(name="psm", bufs=1, space="PSUM")
            psm = psm_att_cm.__enter__()
            psm_proj_cm = tc.tile_pool(name="psmp", bufs=1, space="PSUM")
            psm_proj = psm_proj_cm.__enter__()

            def do_proj(it):
                xT = pm.tile([128, CB, 512], f32r, tag="xT", bufs=2, name="xT")
                src = x_in.rearrange("(cb p) t -> p cb t", p=128)
                # split by cb so the first accumulation matmuls start after
                # only half the strip's x has landed
                for hf in range(3):
                    nc.sync.dma_start(
                        out=xT[:, 2 * hf : 2 * (hf + 1), :],
                        in_=src[:, 2 * hf : 2 * (hf + 1), 512 * it : 512 * (it + 1)],
                    )
                if it == 0:
                    # V weights ride sync AFTER strip 0's x but BEFORE the
                    # psV matmuls are issued (issue order defines deps)
                    load_v_weights()
                # head-2 q/k tail FIRST (attn(2, iq) runs right after this
                # projection): q2 rows 0-63, k2 rows 64-127 in one matmul set
                psB = psm_proj.tile([128, 512], f32, tag="proj", bufs=2, name="psB")
                for cb in range(CB):
                    nc.tensor.matmul(
                        psB, wqk_t[:, cb, :], xT[:, cb, :],
                        start=(cb == 0), stop=(cb == CB - 1),
                    )
                stgB = pm.tile([128, 512], f8, tag="stg", bufs=3, name="stgB")
                nc.vector.tensor_scalar_add(stgB, psB, bqk2)
                nc.gpsimd.dma_start(
                    out=q_f8[2][:, :, 512 * it : 512 * (it + 1)], in_=stgB[0:64, :]
                )
                nc.gpsimd.dma_start(
                    out=k_f8[2][:, :, 512 * it : 512 * (it + 1)], in_=stgB[64:128, :]
                )
                for w_r, bc, dsts in (
                    (wq_r, bq_c, (q_f8[0], q_f8[1])),
                    (wk_r, bk_c, (k_f8[0], k_f8[1])),
                ):
                    psA = psm_proj.tile([128, 512], f32, tag="proj", bufs=2, name="psA")
                    for cb in range(CB):
                        nc.tensor.matmul(
                            psA, w_r[:, cb, 0:128], xT[:, cb, :],
                            start=(cb == 0), stop=(cb == CB - 1),
                        )
                    stg = pm.tile([128, 512], f8, tag="stg", bufs=3, name="stg")
                    nc.vector.tensor_scalar_add(stg, psA, bc)
                    for hh in range(2):
                        nc.gpsimd.dma_start(
                            out=dsts[hh][:, :, 512 * it : 512 * (it + 1)],
                            in_=stg[64 * hh : 64 * (hh + 1), :],
                        )
                for tb in range(4):
                    psV = psm_proj.tile([128, 256], f32, tag="proj", bufs=2, name="psV")
                    for cb in range(CB):
                        nc.tensor.matmul(
                            psV, xT[:, cb, 128 * tb : 128 * (tb + 1)], wv_r[:, cb, :],
                            start=(cb == 0), stop=(cb == CB - 1),
                        )
                    tk = 4 * it + tb
                    nc.vector.tensor_add(
                        v1[:, tk, :, 0:D],
                        psV[:, 0:MPC].rearrange("p (h d) -> p h d", h=HPC),
                        bv_b.rearrange("p (h d) -> p h d", h=HPC),
                    )
                return xT

            def do_attn(h, iq):
                qh, kh = q_f8[h], k_f8[h]
                ps_o = psm.tile([65, 512], f32, tag="o", bufs=2, name="ps_o")
                qs = qh[:, :, 512 * iq : 512 * (iq + 1)]
                # full tk blocks in pairs: one [128, 1024] exp, no masking
                for p in range(2 * iq):
                    ik0, ik1 = 2 * p, 2 * p + 1
                    ps2 = psm.tile([128, 1024], f32, tag="s", bufs=2, name="ps2")
                    nc.tensor.matmul(
                        ps2[:, 0:512], kh[:, :, 128 * ik0 : 128 * (ik0 + 1)], qs,
                        start=True, stop=True, perf_mode=DR,
                    )
                    nc.tensor.matmul(
                        ps2[:, 512:1024], kh[:, :, 128 * ik1 : 128 * (ik1 + 1)], qs,
                        start=True, stop=True, perf_mode=DR,
                    )
                    pT = pm.tile([128, 1024], bf16, tag="pT", bufs=3, name="pT")
                    if DVE_EXP_MOD and (p + iq + h) % DVE_EXP_MOD == 1:
                        yi = pm.tile([128, 1024], i32, tag="yi", bufs=2, name="yi")
                        nc.vector.tensor_scalar(
                            yi, ps2, EXP_A, EXP_B,
                            mybir.AluOpType.mult, mybir.AluOpType.add,
                        )
                        nc.vector.tensor_copy(pT, yi[:, :].bitcast(f32))
                    else:
                        nc.scalar.activation(pT, ps2, EXP, scale=0.125)
                    nc.tensor.matmul(
                        ps_o, v1[:, ik0, h, :], pT[:, 0:512],
                        start=(ik0 == 0), stop=False,
                    )
                    nc.tensor.matmul(
                        ps_o, v1[:, ik1, h, :], pT[:, 512:1024],
                        start=False, stop=False,
                    )
                # diagonal region: 4 causal blocks, TWO exps via paired tiles
                # (j0 full + j1 full share one [128,1024] exp; j2+j3 share a
                # strided-AP exp over their two 256-col regions)
                ik0 = 4 * iq
                qs_c = qh[:, :, 512 * iq : 512 * (iq + 1)]
                ps2a = psm.tile([128, 1024], f32, tag="s", bufs=2, name="ps2")
                nc.tensor.matmul(
                    ps2a[:, 0:512], kh[:, :, 128 * ik0 : 128 * (ik0 + 1)],
                    qs_c, start=True, stop=True, perf_mode=DR,
                )
                nc.tensor.matmul(
                    ps2a[:, 512:1024], kh[:, :, 128 * (ik0 + 1) : 128 * (ik0 + 2)],
                    qs_c, start=True, stop=True, perf_mode=DR,
                )
                pTa = pm.tile([128, 1024], bf16, tag="pT", bufs=3, name="pT")
                nc.scalar.activation(pTa, ps2a, EXP, scale=0.125)
                nc.vector.tensor_mul(pTa[:, 0:128], pTa[:, 0:128], mask[:, 128:256])
                nc.vector.tensor_mul(pTa[:, 512:768], pTa[:, 512:768], mask)
                nc.tensor.matmul(
                    ps_o, v1[:, ik0, h, :], pTa[:, 0:512],
                    start=(ik0 == 0), stop=False,
                )
                nc.tensor.matmul(
                    ps_o[:, 128:], v1[:, ik0 + 1, h, :], pTa[:, 640:1024],
                    start=False, stop=False,
                )
                ps2b = psm.tile([128, 1024], f32, tag="s", bufs=2, name="ps2")
                qs_d = qh[:, :, 512 * iq + 256 : 512 * (iq + 1)]
                nc.tensor.matmul(
                    ps2b[:, 256:512], kh[:, :, 128 * (ik0 + 2) : 128 * (ik0 + 3)],
                    qs_d, start=True, stop=True, perf_mode=DR,
                )
                nc.tensor.matmul(
                    ps2b[:, 768:1024], kh[:, :, 128 * (ik0 + 3) : 128 * (ik0 + 4)],
                    qs_d, start=True, stop=True, perf_mode=DR,
                )
                pTb = pm.tile([128, 1024], bf16, tag="pT", bufs=3, name="pT")
                pTb_v = bass.AP(
                    tensor=pTb.tensor, offset=pTb[:, 256:512].offset,
                    ap=[list(pTb[:, 256:512].ap[0]), [512, 2], [1, 256]],
                )
                ps2b_v = bass.AP(
                    tensor=ps2b.tensor, offset=ps2b[:, 256:512].offset,
                    ap=[list(ps2b[:, 256:512].ap[0]), [512, 2], [1, 256]],
                )
                nc.scalar.activation(pTb_v, ps2b_v, EXP, scale=0.125)
                nc.vector.tensor_mul(pTb[:, 256:384], pTb[:, 256:384], mask[:, 128:256])
                nc.vector.tensor_mul(pTb[:, 768:1024], pTb[:, 768:1024], mask)
                nc.tensor.matmul(
                    ps_o[:, 256:], v1[:, ik0 + 2, h, :], pTb[:, 256:512],
                    start=False, stop=False,
                )
                nc.tensor.matmul(
                    ps_o[:, 256:], v1[:, ik0 + 3, h, :], pTb[:, 768:1024],
                    start=False, stop=True,
                )
                att_c = pm.tile([65, 512], bf16, tag="an", bufs=3, name="att_c")
                nc.vector.tensor_copy(att_c, ps_o)
                nc.sync.dma_start(out=a2a_in[h][iq, :, :], in_=att_c)
                return att_c

            def do_a2a(h):
                nc.gpsimd.collective_compute(
                    "AllToAll",
                    mybir.AluOpType.bypass,
                    replica_groups=[list(range(NCORES))],
                    ins=[a2a_in[h][:]],
                    outs=[a2a_out[h][:]],
                )

            def phase3(h, stage, psm_p3, gate=None):
                # gathered rows for head h: flats rows 64*src + d; chunk
                # (bb, half) = rows 256bb+128half.. = a2a_out[h][4bb+2half:+2]
                ltbs = []
                for bb in range(2):
                    for half in range(2):
                        s0 = 4 * bb + 2 * half
                        ltb = pm.tile([128, 512], bf16, tag="ltb", bufs=4, name="ltb")
                        if gate is not None:
                            # WAW token: the scheduling pass underestimates
                            # collective latency and would otherwise slot this
                            # chunk's matmuls ahead of remaining attention,
                            # head-of-line-blocking the PE queue on the
                            # collective. Tying the ltb tile to a late
                            # attention tile forces the chunk after that work.
                            nc.vector.tensor_copy(ltb[64:65, 0:1], gate[64:65, 0:1])
                        ltb_eng = (nc.gpsimd, nc.sync)[half] if stage == 2 else nc.gpsimd
                        ltb_eng.dma_start(
                            out=ltb, in_=a2a_out[h][s0 : s0 + 2, 0:D, :]
                        )
                        # denominators: row 64 of each src, partition-broadcast
                        # from DRAM (64 copies each) into [128, 512]; in the
                        # final chunk they ride the idle ACT queue
                        den = pm.tile([128, 512], bf16, tag="den", bufs=4, name="den")
                        dsrc = a2a_out[h][s0 : s0 + 2, D : D + 1, :]
                        den_eng = nc.scalar if stage == 2 else nc.gpsimd
                        den_eng.dma_start(
                            out=den,
                            in_=bass.AP(
                                tensor=dsrc.tensor,
                                offset=dsrc.offset,
                                ap=[list(dsrc.ap[0]), [0, D], [1, 512]],
                            ),
                        )
                        # DVE has no divide ISA op: reciprocal then multiply
                        dre = pm.tile([128, 512], f32, tag="dre", bufs=4, name="dre")
                        nc.vector.reciprocal(dre, den)
                        ltn = pm.tile([128, 512], f32r, tag="ltn", bufs=4, name="ltn")
                        with nc.allow_low_precision(reason="f32r stores f32 bits"):
                            nc.vector.tensor_mul(ltn, ltb, dre)
                        ltbs.append(ltn)
                for bb in range(2):
                    for tb in range(4):
                        # one [128, 768] PSUM tile: cols 0-511 in bank A,
                        # 512-767 in bank B, separate accumulation groups.
                        # The final chunk runs after the attention PSUM pool
                        # closes, so it can triple-buffer.
                        ps = psm_p3.tile(
                            [128, C], f32, tag="pa",
                            bufs=3 if stage == 2 else 1, name="pa",
                        )
                        for half in range(2):
                            lt = ltbs[2 * bb + half][:, 128 * tb : 128 * (tb + 1)]
                            nc.tensor.matmul(
                                ps[:, 0:512], lt, wo_r[:, 2 * h + half, 0:512],
                                start=(half == 0), stop=(half == 1 and stage != 2),
                            )
                            nc.tensor.matmul(
                                ps[:, 512:C], lt, wo_r[:, 2 * h + half, 512:C],
                                start=(half == 0), stop=(half == 1 and stage != 2),
                            )
                        a = acc[:, bb, tb, :]
                        if stage == 0:
                            with nc.allow_low_precision(reason="f32r=f32 bits"):
                                nc.vector.tensor_add(a, ps, bo_b)
                        elif stage == 1:
                            with nc.allow_low_precision(reason="f32r=f32 bits"):
                                nc.vector.tensor_add(a, a, ps)
                        else:
                            # final stage: fold acc into the PSUM group with an
                            # identity matmul, then evacuate PSUM->SBUF with
                            # copies alternating DVE / idle ACT, and DMA out on
                            # two queues (PSUM can't be a DMA source)
                            nc.tensor.matmul(
                                ps[:, 0:512], ident, a[:, 0:512],
                                start=False, stop=True,
                            )
                            nc.tensor.matmul(
                                ps[:, 512:C], ident, a[:, 512:C],
                                start=False, stop=True,
                            )
                            out_t = pm.tile([128, C], f32, tag="ot", bufs=4, name="out_t")
                            if (bb * 4 + tb) % 2 == 0:
                                nc.vector.tensor_copy(out_t, ps)
                                dma_eng = nc.sync
                            else:
                                nc.scalar.activation(
                                    out_t, ps,
                                    mybir.ActivationFunctionType.Copy,
                                )
                                dma_eng = nc.gpsimd
                            dma_eng.dma_start(
                                out=out_d[bb, 128 * tb : 128 * (tb + 1), :], in_=out_t
                            )

            # strips 0-3: proj + all heads (head 2 first so its tail is early)
            for iq in range(4):
                do_proj(iq)
                for h in (2, 1, 0):
                    do_attn(h, iq)
            # strips 4-7: head 2 rides along the remaining projections; wo's
            # big load slips in here (gpsimd queue is past the strip-0 folds)
            for iq in range(4, NQB):
                xT_iq = do_proj(iq)
                do_attn(2, iq)
                if iq in (4, 5, 6):
                    # wo in 2-cb chunks on SP: off the Pool queue (collectives
                    # + folds). WAW-gated on this strip's xT so the scheduler
                    # can't hoist them ahead of the x loads.
                    c0 = 2 * (iq - 4)
                    nc.vector.tensor_copy(wo_r[0:1, c0, 0:1], xT_iq[0:1, 0, 0:1])
                    nc.sync.dma_start(
                        out=wo_r[:, c0 : c0 + 2, :],
                        in_=wo_in.rearrange("(cb p) m -> p cb m", p=128)[
                            :, c0 : c0 + 2, :
                        ],
                    )
            psm_proj_cm.__exit__(None, None, None)
            psm_p3_cm = tc.tile_pool(name="psm3", bufs=1, space="PSUM")
            psm_p3 = psm_p3_cm.__enter__()
            do_a2a(2)
            for iq in range(4, NQB):
                g1 = do_attn(1, iq)
                if iq == 6:
                    gate_b = g1
            do_a2a(1)
            do_attn(0, 4)
            phase3(2, 0, psm_p3, gate=gate_b)
            do_attn(0, 5)
            g0 = do_attn(0, 6)
            do_attn(0, 7)
            do_a2a(0)
            phase3(1, 1, psm_p3, gate=g0)
            # attention is done: free its 6 PSUM banks (LIFO pool order) so
            # the last Wo chunk — the only thing left on the critical path —
            # can triple-buffer its accumulators
            psm_p3_cm.__exit__(None, None, None)
            psm_att_cm.__exit__(None, None, None)
            psm_p3b_cm = tc.tile_pool(name="psm3b", bufs=1, space="PSUM")
            psm_p3b = psm_p3b_cm.__enter__()
            phase3(0, 2, psm_p3b)
            psm_p3b_cm.__exit__(None, None, None)

    nc.finalize()
    return nc


def kernel(x, Wq, bq, Wk, bk, Wv, bv, Wo, bo):
    if "nc" not in _CACHE:
        _CACHE["nc"] = _build()
    nc = _CACHE["nc"]

    x = np.asarray(x, dtype=np.float32)
    # permute Wo rows from global head-dim order (192g + 64h + d) to the
    # head-major gathered layout (256h + 64g + d) used by phase 3
    perm = np.empty(C, dtype=np.int64)
    for h_l in range(HPC):
        for g in range(4):
            perm[256 * h_l + 64 * g : 256 * h_l + 64 * g + 64] = np.arange(
                MPC * g + D * h_l, MPC * g + D * h_l + D
            )
    wo_send = np.ascontiguousarray(np.asarray(Wo, np.float32)[perm, :])
    in_maps = []
    for c in range(NCORES):
        b, g = c // 4, c % 4
        sl = slice(MPC * g, MPC * (g + 1))
        in_maps.append({
            "x": np.ascontiguousarray(x[b].T),
            "wq": np.ascontiguousarray(np.asarray(Wq, np.float32)[:, sl]),
            "wk": np.ascontiguousarray(np.asarray(Wk, np.float32)[:, sl]),
            "wv": np.ascontiguousarray(np.asarray(Wv, np.float32)[:, sl]),
            "bq": np.ascontiguousarray(np.asarray(bq, np.float32)[sl]),
            "bk": np.ascontiguousarray(np.asarray(bk, np.float32)[sl]),
            "bv": np.ascontiguousarray(np.asarray(bv, np.float32)[sl]),
            "wo": wo_send,
            "bo": np.ascontiguousarray(np.asarray(bo, np.float32)),
        })

    res = run_bass_kernel_spmd(nc, in_maps, core_ids=list(range(NCORES)))
    out = np.empty((2, T, C), dtype=np.float32)
    for j in range(NCORES):
        r = res.results[j]["out"]
        out[0, 512 * j : 512 * (j + 1), :] = r[0]
        out[1, 512 * j : 512 * (j + 1), :] = r[1]
    return out


# revision 55
# speedup vs baseline: 1.2979x; 1.0061x over previous
"""Causal multi-head attention (B=2, T=4096, C=768, H=12) on 8 Trainium2 cores.

Sharding: core c handles batch b=c//4 and heads 3*(c%4)..3*(c%4)+2 for the
QKV projections and flash attention; one 8-way AllToAll PER HEAD redistributes
that head's attention output (bf16) so core j holds ALL heads for tq strip j
(both batches); each core then runs the Wo projection for its 2x512 output
rows, one per-head chunk at a time as each head's AllToAll lands.

Key optimizations vs the f32r baseline:
- x arrives HOST-PRETRANSPOSED as [C, T], so the per-strip PE transposes and
  PSUM->SBUF copies are gone; projections read xT strips straight from DRAM.
- Scores matmuls run in fp8e4m3 with DoubleRow perf mode (0.5 cyc/row): q/k
  are stored as [32, 2, T] fp8 tiles (contraction dim d=2p+i folded onto 32
  partitions x 2 column blocks) via one staging fp8 bias-add + one fold DMA.
- Everything downstream of exp is bf16 (pT, V, attention outputs), halving
  the per-head AllToAll to 512KB.
- Head-rotated schedule: head 2 finishes its strips first and its AllToAll
  fires ~90us before the end; phase-3 Wo chunks are interleaved per head so
  only the last head's 28us collective + 5us Wo chunk trail the main loop.
Flash attention uses no-max-subtraction softmax (scores are O(+-5), exp is
safe) with the denominator computed by an appended ones-column on V.
"""
import numpy as np
from contextlib import ExitStack

import concourse.bass as bass
import concourse.mybir as mybir
import concourse.tile as tile
from concourse import bacc
from concourse.bass_utils import run_bass_kernel_spmd
from concourse.masks import make_identity, make_upper_triangular

T = 4096
C = 768
H = 12
D = 64
HPC = 3            # heads per core
MPC = HPC * D      # 192 projected dims per core
NCORES = 8
NTB = T // 128     # 32 tk blocks
NQB = T // 512     # 8 tq strips
CB = C // 128      # 6 contraction blocks
f32 = mybir.dt.float32
f32r = mybir.dt.float32r
bf16 = mybir.dt.bfloat16
f8 = mybir.dt.float8e4
EXP = mybir.ActivationFunctionType.Exp
DR = mybir.MatmulPerfMode.DoubleRow
i32 = mybir.dt.int32

# Schraudolph exp-approximation on DVE for a fraction of the score blocks:
# exp(s/8) ~= bitcast_f32(int32(s*EXP_A + EXP_B)). Offloads the ACT engine
# (the critical engine) onto idle DVE cycles. ~1.8% RMS per-weight error on
# the routed blocks; numerator and denominator use the same approximation so
# softmax row sums stay exact.
EXP_A = float(0.125 * (1 << 23) / np.log(2.0))
EXP_B = float(127 * (1 << 23) - 486408 + 0.5)
DVE_EXP_MOD = 0  # disabled: DVE detour stalls the in-order PV chain, and the
                 # ~2% weight noise would exhaust the 2e-2 error budget on top
                 # of fp8 scores (measured 1.82e-2 with fp8 alone)

_CACHE = {}


def _build():
    nc = bacc.Bacc(None, target_bir_lowering=False, num_devices=NCORES)
    # x is pre-transposed on the host: [C, T] for this core's batch
    x_in = nc.declare_dram_parameter("x", [C, T], f32r, isOutput=False)
    wq_in = nc.declare_dram_parameter("wq", [C, MPC], f32r, isOutput=False)
    wk_in = nc.declare_dram_parameter("wk", [C, MPC], f32r, isOutput=False)
    wv_in = nc.declare_dram_parameter("wv", [C, MPC], f32r, isOutput=False)
    bq_in = nc.declare_dram_parameter("bq", [MPC], f32, isOutput=False)
    bk_in = nc.declare_dram_parameter("bk", [MPC], f32, isOutput=False)
    bv_in = nc.declare_dram_parameter("bv", [MPC], f32, isOutput=False)
    wo_in = nc.declare_dram_parameter("wo", [C, C], f32r, isOutput=False)
    bo_in = nc.declare_dram_parameter("bo", [C], f32, isOutput=False)
    out_d = nc.declare_dram_parameter("out", [2, 512, C], f32, isOutput=True)

    with tile.TileContext(nc) as tc, ExitStack() as ctx:
        singles = ctx.enter_context(tc.tile_pool(name="singles", bufs=1))
        dram = ctx.enter_context(tc.tile_pool(name="dram", bufs=1, space="DRAM"))

        # ---- static tiles -------------------------------------------------
        # mask[:, 0:128] = 0, mask[:, 128:256] = upper-tri (c >= r), bf16
        mask = singles.tile([128, 256], bf16)
        nc.gpsimd.memset(mask[:, 0:128], 0.0)
        make_upper_triangular(nc, mask[:, 128:256], val=1.0)
        # identity for the phase-3 final accumulate (acc folded into PSUM by
        # an extra matmul so the tail needs no DVE adds)
        ident = singles.tile([128, 128], f32r)
        with tc.tile_pool(name="idstage", bufs=1) as idstage:
            idf = idstage.tile([128, 128], f32)
            make_identity(nc, idf)
            nc.vector.tensor_copy(ident, idf)

        # ---- weights -> SBUF (f32r params: straight DMA, no rounding copies)
        wq_r = singles.tile([128, CB, MPC], f32r)
        wk_r = singles.tile([128, CB, MPC], f32r)
        # wv padded to 256 free cols (zeros) so the v-proj matmul has N=256
        wv_r = singles.tile([128, CB, 256], f32r)
        wo_r = singles.tile([128, CB, C], f32r)
        # load order matters: attn(2, 0) is the first attention work and needs
        # wqk_t (its q/k come from the psB tail matmuls), so wqk_t loads
        # before wq/wk; wo's 2.4MB load is deferred into the main loop
        # combined q-tail/k-tail weight: one [128, 512] projection matmul set
        # yields q2 rows 0-63 and k2 rows 64-127
        wqk_t = singles.tile([128, CB, 128], f32r)
        nc.gpsimd.dma_start(
            out=wqk_t[:, :, 0:64],
            in_=wq_in.rearrange("(cb p) m -> p cb m", p=128)[:, :, 128:MPC],
        )
        nc.gpsimd.dma_start(
            out=wqk_t[:, :, 64:128],
            in_=wk_in.rearrange("(cb p) m -> p cb m", p=128)[:, :, 128:MPC],
        )
        nc.gpsimd.dma_start(out=wq_r, in_=wq_in.rearrange("(cb p) m -> p cb m", p=128))
        nc.gpsimd.dma_start(out=wk_r, in_=wk_in.rearrange("(cb p) m -> p cb m", p=128))
        # wv/bv/bo ride the sync queue inside the main loop so strip-0's q/k
        # fold DMAs aren't queued behind them on gpsimd (V work isn't needed
        # until the first PV matmul, well after the first scores+exp)
        with tc.tile_pool(name="wstage", bufs=1) as wstage:
            zpad = wstage.tile([128, CB, 64], f32)
            nc.vector.memset(zpad, 0.0)
            nc.vector.tensor_copy(wv_r[:, :, MPC:256], zpad)
            # pre-trigger the exp table load so its ~2.7us hides in the ramp
            warm = wstage.tile([1, 2], f32)
            nc.vector.memset(warm, 0.0)
            nc.scalar.activation(warm[:, 1:2], warm[:, 0:1], EXP, scale=1.0)

        # ---- biases: tiny loads on the idle ACT queue, so the sync queue
        # carries nothing but x strips during the ramp ----------------------
        bq_c = singles.tile([128, 1], f32)
        bk_c = singles.tile([128, 1], f32)
        # combined tail bias: rows 0-63 = bq[128:192], rows 64-127 = bk[128:192]
        bqk2 = singles.tile([128, 1], f32)
        nc.scalar.dma_start(out=bqk2[0:64, :], in_=bq_in[128:MPC].unsqueeze(1))
        nc.scalar.dma_start(out=bqk2[64:128, :], in_=bk_in[128:MPC].unsqueeze(1))
        nc.scalar.dma_start(out=bq_c, in_=bq_in[0:128].unsqueeze(1))
        nc.scalar.dma_start(out=bk_c, in_=bk_in[0:128].unsqueeze(1))
        bv_b = singles.tile([128, MPC], f32)
        bo_b = singles.tile([128, C], f32)

        def load_v_weights():
            # the ACT queue is idle during the ramp; using it keeps these off
            # the SP queue (x strips) and Pool queue (q/k folds)
            nc.scalar.dma_start(
                out=wv_r[:, :, 0:MPC], in_=wv_in.rearrange("(cb p) m -> p cb m", p=128)
            )
            nc.scalar.dma_start(
                out=bv_b,
                in_=bass.AP(tensor=bv_in.ap().tensor, offset=0, ap=[[0, 128]] + bv_in.ap().ap),
            )
            nc.scalar.dma_start(
                out=bo_b,
                in_=bass.AP(tensor=bo_in.ap().tensor, offset=0, ap=[[0, 128]] + bo_in.ap().ap),
            )

        # ---- persistent activation buffers --------------------------------
        # q/k per head in fp8, folded for DoubleRow: [32, 2, T], d = 2p+i
        q_f8 = tuple(singles.tile([32, 2, T], f8, name=f"qf8_{h}") for h in range(HPC))
        k_f8 = tuple(singles.tile([32, 2, T], f8, name=f"kf8_{h}") for h in range(HPC))
        # V + ones column, per tk block and head: [128, 32, 3, 65] bf16
        v1 = singles.tile([128, NTB, HPC, D + 1], bf16)
        ones_t = singles.tile([128, NTB, HPC], bf16)
        nc.vector.memset(ones_t, 1.0)
        nc.vector.tensor_copy(v1[:, :, :, D], ones_t)
        # phase-3 SBUF accumulator: [128 tokens, bb, tb, C]; f32r so the
        # final chunk can feed it back through the PE as a moving operand
        acc = singles.tile([128, 2, 4, C], f32r)

        # one tile pair per head: head h's AllToAll fires as soon as that
        # head's attention finishes. Row 64 carries the UNNORMALIZED softmax
        # denominator; normalization happens receiver-side in phase 3, so the
        # per-strip critical chain is one bf16 copy + one DMA.
        a2a_in = tuple(
            dram.tile([NCORES, D + 1, 512], bf16, name=f"a2a_in{h}") for h in range(HPC)
        )
        a2a_out = tuple(
            dram.tile([NCORES, D + 1, 512], bf16, name=f"a2a_out{h}") for h in range(HPC)
        )

        # ---- main loop ----------------------------------------------------
        with tc.tile_pool(name="pm", bufs=1) as pm:
            psm_att_cm = tc.tile_pool(name="psm", bufs=1, space="PSUM")
            psm = psm_att_cm.__enter__()
            psm_proj_cm = tc.tile_pool(name="psmp", bufs=1, space="PSUM")
            psm_proj = psm_proj_cm.__enter__()

            def do_proj(it):
                xT = pm.tile([128, CB, 512], f32r, tag="xT", bufs=2, name="xT")
                src = x_in.rearrange("(cb p) t -> p cb t", p=128)
                # split by cb so the first accumulation matmuls start after
                # only half the strip's x has landed
                for hf in range(3):
                    nc.sync.dma_start(
                        out=xT[:, 2 * hf : 2 * (hf + 1), :],
                        in_=src[:, 2 * hf : 2 * (hf + 1), 512 * it : 512 * (it + 1)],
                    )
                if it == 0:
                    # V weights ride sync AFTER strip 0's x but BEFORE the
                    # psV matmuls are issued (issue order defines deps)
                    load_v_weights()
                # head-2 q/k tail FIRST (attn(2, iq) runs right after this
                # projection): q2 rows 0-63, k2 rows 64-127 in one matmul set
                psB = psm_proj.tile([128, 512], f32, tag="proj", bufs=2, name="psB")
                for cb in range(CB):
                    nc.tensor.matmul(
                        psB, wqk_t[:, cb, :], xT[:, cb, :],
                        start=(cb == 0), stop=(cb == CB - 1),
                    )
                stgB = pm.tile([128, 512], f8, tag="stg", bufs=3, name="stgB")
                nc.vector.tensor_scalar_add(stgB, psB, bqk2)
                nc.gpsimd.dma_start(
                    out=q_f8[2][:, :, 512 * it : 512 * (it + 1)], in_=stgB[0:64, :]
                )
                nc.gpsimd.dma_start(
                    out=k_f8[2][:, :, 512 * it : 512 * (it + 1)], in_=stgB[64:128, :]
                )
                for w_r, bc, dsts in (
                    (wq_r, bq_c, (q_f8[0], q_f8[1])),
                    (wk_r, bk_c, (k_f8[0], k_f8[1])),
                ):
                    psA = psm_proj.tile([128, 512], f32, tag="proj", bufs=2, name="psA")
                    for cb in range(CB):
                        nc.tensor.matmul(
                            psA, w_r[:, cb, 0:128], xT[:, cb, :],
                            start=(cb == 0), stop=(cb == CB - 1),
                        )
                    stg = pm.tile([128, 512], f8, tag="stg", bufs=3, name="stg")
                    nc.vector.tensor_scalar_add(stg, psA, bc)
                    for hh in range(2):
                        nc.gpsimd.dma_start(
                            out=dsts[hh][:, :, 512 * it : 512 * (it + 1)],
                            in_=stg[64 * hh : 64 * (hh + 1), :],
                        )
                for tb in range(4):
                    psV = psm_proj.tile([128, 256], f32, tag="proj", bufs=2, name="psV")
                    for cb in range(CB):
                        nc.tensor.matmul(
                            psV, xT[:, cb, 128 * tb : 128 * (tb + 1)], wv_r[:, cb, :],
                            start=(cb == 0), stop=(cb == CB - 1),
                        )
                    tk = 4 * it + tb
                    nc.vector.tensor_add(
                        v1[:, tk, :, 0:D],
                        psV[:, 0:MPC].rearrange("p (h d) -> p h d", h=HPC),
                        bv_b.rearrange("p (h d) -> p h d", h=HPC),
                    )
                return xT

            def do_attn(h, iq):
                qh, kh = q_f8[h], k_f8[h]
                ps_o = psm.tile([65, 512], f32, tag="o", bufs=2, name="ps_o")
                qs = qh[:, :, 512 * iq : 512 * (iq + 1)]
                # full tk blocks in pairs: one [128, 1024] exp, no masking
                for p in range(2 * iq):
                    ik0, ik1 = 2 * p, 2 * p + 1
                    ps2 = psm.tile([128, 1024], f32, tag="s", bufs=2, name="ps2")
                    nc.tensor.matmul(
                        ps2[:, 0:512], kh[:, :, 128 * ik0 : 128 * (ik0 + 1)], qs,
                        start=True, stop=True, perf_mode=DR,
                    )
                    nc.tensor.matmul(
                        ps2[:, 512:1024], kh[:, :, 128 * ik1 : 128 * (ik1 + 1)], qs,
                        start=True, stop=True, perf_mode=DR,
                    )
                    pT = pm.tile([128, 1024], bf16, tag="pT", bufs=3, name="pT")
                    if DVE_EXP_MOD and (p + iq + h) % DVE_EXP_MOD == 1:
                        yi = pm.tile([128, 1024], i32, tag="yi", bufs=2, name="yi")
                        nc.vector.tensor_scalar(
                            yi, ps2, EXP_A, EXP_B,
                            mybir.AluOpType.mult, mybir.AluOpType.add,
                        )
                        nc.vector.tensor_copy(pT, yi[:, :].bitcast(f32))
                    else:
                        nc.scalar.activation(pT, ps2, EXP, scale=0.125)
                    nc.tensor.matmul(
                        ps_o, v1[:, ik0, h, :], pT[:, 0:512],
                        start=(ik0 == 0), stop=False,
                    )
                    nc.tensor.matmul(
                        ps_o, v1[:, ik1, h, :], pT[:, 512:1024],
                        start=False, stop=False,
                    )
                # diagonal region: 4 causal blocks, TWO exps via paired tiles
                # (j0 full + j1 full share one [128,1024] exp; j2+j3 share a
                # strided-AP exp over their two 256-col regions)
                ik0 = 4 * iq
                qs_c = qh[:, :, 512 * iq : 512 * (iq + 1)]
                ps2a = psm.tile([128, 1024], f32, tag="s", bufs=2, name="ps2")
                nc.tensor.matmul(
                    ps2a[:, 0:512], kh[:, :, 128 * ik0 : 128 * (ik0 + 1)],
                    qs_c, start=True, stop=True, perf_mode=DR,
                )
                nc.tensor.matmul(
                    ps2a[:, 512:1024], kh[:, :, 128 * (ik0 + 1) : 128 * (ik0 + 2)],
                    qs_c, start=True, stop=True, perf_mode=DR,
                )
                pTa = pm.tile([128, 1024], bf16, tag="pT", bufs=3, name="pT")
                nc.scalar.activation(pTa, ps2a, EXP, scale=0.125)
                nc.vector.tensor_mul(pTa[:, 0:128], pTa[:, 0:128], mask[:, 128:256])
                nc.vector.tensor_mul(pTa[:, 512:768], pTa[:, 512:768], mask)
                nc.tensor.matmul(
                    ps_o, v1[:, ik0, h, :], pTa[:, 0:512],
                    start=(ik0 == 0), stop=False,
                )
                nc.tensor.matmul(
                    ps_o[:, 128:], v1[:, ik0 + 1, h, :], pTa[:, 640:1024],
                    start=False, stop=False,
                )
                ps2b = psm.tile([128, 1024], f32, tag="s", bufs=2, name="ps2")
                qs_d = qh[:, :, 512 * iq + 256 : 512 * (iq + 1)]
                nc.tensor.matmul(
                    ps2b[:, 256:512], kh[:, :, 128 * (ik0 + 2) : 128 * (ik0 + 3)],
                    qs_d, start=True, stop=True, perf_mode=DR,
                )
                nc.tensor.matmul(
                    ps2b[:, 768:1024], kh[:, :, 128 * (ik0 + 3) : 128 * (ik0 + 4)],
                    qs_d, start=True, stop=True, perf_mode=DR,
                )
                pTb = pm.tile([128, 1024], bf16, tag="pT", bufs=3, name="pT")
                pTb_v = bass.AP(
                    tensor=pTb.tensor, offset=pTb[:, 256:512].offset,
                    ap=[list(pTb[:, 256:512].ap[0]), [512, 2], [1, 256]],
                )
                ps2b_v = bass.AP(
                    tensor=ps2b.tensor, offset=ps2b[:, 256:512].offset,
                    ap=[list(ps2b[:, 256:512].ap[0]), [512, 2], [1, 256]],
                )
                nc.scalar.activation(pTb_v, ps2b_v, EXP, scale=0.125)
                nc.vector.tensor_mul(pTb[:, 256:384], pTb[:, 256:384], mask[:, 128:256])
                nc.vector.tensor_mul(pTb[:, 768:1024], pTb[:, 768:1024], mask)
                nc.tensor.matmul(
                    ps_o[:, 256:], v1[:, ik0 + 2, h, :], pTb[:, 256:512],
                    start=False, stop=False,
                )
                nc.tensor.matmul(
                    ps_o[:, 256:], v1[:, ik0 + 3, h, :], pTb[:, 768:1024],
                    start=False, stop=True,
                )
                att_c = pm.tile([65, 512], bf16, tag="an", bufs=3, name="att_c")
                nc.vector.tensor_copy(att_c, ps_o)
                nc.sync.dma_start(out=a2a_in[h][iq, :, :], in_=att_c)
                return att_c

            def do_a2a(h):
                nc.gpsimd.collective_compute(
                    "AllToAll",
                    mybir.AluOpType.bypass,
                    replica_groups=[list(range(NCORES))],
                    ins=[a2a_in[h][:]],
                    outs=[a2a_out[h][:]],
                )

            def phase3(h, stage, psm_p3, gate=None):
                # gathered rows for head h: flats rows 64*src + d; chunk
                # (bb, half) = rows 256bb+128half.. = a2a_out[h][4bb+2half:+2]
                ltbs = []
                for bb in range(2):
                    for half in range(2):
                        s0 = 4 * bb + 2 * half
                        ltb = pm.tile([128, 512], bf16, tag="ltb", bufs=4, name="ltb")
                        if gate is not None:
                            # WAW token: the scheduling pass underestimates
                            # collective latency and would otherwise slot this
                            # chunk's matmuls ahead of remaining attention,
                            # head-of-line-blocking the PE queue on the
                            # collective. Tying the ltb tile to a late
                            # attention tile forces the chunk after that work.
                            nc.vector.tensor_copy(ltb[64:65, 0:1], gate[64:65, 0:1])
                        ltb_eng = (nc.gpsimd, nc.sync)[half] if stage == 2 else nc.gpsimd
                        ltb_eng.dma_start(
                            out=ltb, in_=a2a_out[h][s0 : s0 + 2, 0:D, :]
                        )
                        # denominators: row 64 of each src, partition-broadcast
                        # from DRAM (64 copies each) into [128, 512]; in the
                        # final chunk they ride the idle ACT queue
                        den = pm.tile([128, 512], bf16, tag="den", bufs=4, name="den")
                        dsrc = a2a_out[h][s0 : s0 + 2, D : D + 1, :]
                        den_eng = nc.scalar if stage == 2 else nc.gpsimd
                        den_eng.dma_start(
                            out=den,
                            in_=bass.AP(
                                tensor=dsrc.tensor,
                                offset=dsrc.offset,
                                ap=[list(dsrc.ap[0]), [0, D], [1, 512]],
                            ),
                        )
                        # DVE has no divide ISA op: reciprocal then multiply
                        dre = pm.tile([128, 512], f32, tag="dre", bufs=4, name="dre")
                        nc.vector.reciprocal(dre, den)
                        ltn = pm.tile([128, 512], f32r, tag="ltn", bufs=4, name="ltn")
                        with nc.allow_low_precision(reason="f32r stores f32 bits"):
                            nc.vector.tensor_mul(ltn, ltb, dre)
                        ltbs.append(ltn)
                for bb in range(2):
                    for tb in range(4):
                        # one [128, 768] PSUM tile: cols 0-511 in bank A,
                        # 512-767 in bank B, separate accumulation groups.
                        # The final chunk runs after the attention PSUM pool
                        # closes, so it can triple-buffer.
                        ps = psm_p3.tile(
                            [128, C], f32, tag="pa",
                            bufs=3 if stage == 2 else 1, name="pa",
                        )
                        for half in range(2):
                            lt = ltbs[2 * bb + half][:, 128 * tb : 128 * (tb + 1)]
                            nc.tensor.matmul(
                                ps[:, 0:512], lt, wo_r[:, 2 * h + half, 0:512],
                                start=(half == 0), stop=(half == 1 and stage != 2),
                            )
                            nc.tensor.matmul(
                                ps[:, 512:C], lt, wo_r[:, 2 * h + half, 512:C],
                                start=(half == 0), stop=(half == 1 and stage != 2),
                            )
                        a = acc[:, bb, tb, :]
                        if stage == 0:
                            with nc.allow_low_precision(reason="f32r=f32 bits"):
                                nc.vector.tensor_add(a, ps, bo_b)
                        elif stage == 1:
                            with nc.allow_low_precision(reason="f32r=f32 bits"):
                                nc.vector.tensor_add(a, a, ps)
                        else:
                            # final stage: fold acc into the PSUM group with an
                            # identity matmul, then evacuate PSUM->SBUF with
                            # copies alternating DVE / idle ACT, and DMA out on
                            # two queues (PSUM can't be a DMA source)
                            nc.tensor.matmul(
                                ps[:, 0:512], ident, a[:, 0:512],
                                start=False, stop=True,
                            )
                            nc.tensor.matmul(
                                ps[:, 512:C], ident, a[:, 512:C],
                                start=False, stop=True,
                            )
                            out_t = pm.tile([128, C], f32, tag="ot", bufs=4, name="out_t")
                            if (bb * 4 + tb) % 2 == 0:
                                nc.vector.tensor_copy(out_t, ps)
                                dma_eng = nc.sync
                            else:
                                nc.scalar.activation(
                                    out_t, ps,
                                    mybir.ActivationFunctionType.Copy,
                                )
                                dma_eng = nc.gpsimd
                            dma_eng.dma_start(
                                out=out_d[bb, 128 * tb : 128 * (tb + 1), :], in_=out_t
                            )

            # strips 0-3: proj + all heads (head 2 first so its tail is early)
            for iq in range(4):
                do_proj(iq)
                for h in (2, 1, 0):
                    do_attn(h, iq)
            # strips 4-7: head 2 rides along the remaining projections; wo's
            # big load slips in here (gpsimd queue is past the strip-0 folds)
            for iq in range(4, NQB):
                xT_iq = do_proj(iq)
                do_attn(2, iq)
                if iq in (4, 5, 6):
                    # wo in 2-cb chunks on SP: off the Pool queue (collectives
                    # + folds). WAW-gated on this strip's xT so the scheduler
                    # can't hoist them ahead of the x loads.
                    c0 = 2 * (iq - 4)
                    nc.vector.tensor_copy(wo_r[0:1, c0, 0:1], xT_iq[0:1, 0, 0:1])
                    nc.sync.dma_start(
                        out=wo_r[:, c0 : c0 + 2, :],
                        in_=wo_in.rearrange("(cb p) m -> p cb m", p=128)[
                            :, c0 : c0 + 2, :
                        ],
                    )
            psm_proj_cm.__exit__(None, None, None)
            psm_p3_cm = tc.tile_pool(name="psm3", bufs=1, space="PSUM")
            psm_p3 = psm_p3_cm.__enter__()
            do_a2a(2)
            for iq in range(4, NQB):
                g1 = do_attn(1, iq)
                if iq == 6:
                    gate_b = g1
            do_a2a(1)
            do_attn(0, 4)
            phase3(2, 0, psm_p3, gate=gate_b)
            do_attn(0, 5)
            g0 = do_attn(0, 6)
            do_attn(0, 7)
            do_a2a(0)
            phase3(1, 1, psm_p3, gate=g0)
            # attention is done: free its 6 PSUM banks (LIFO pool order) so
            # the last Wo chunk — the only thing left on the critical path —
            # can triple-buffer its accumulators
            psm_p3_cm.__exit__(None, None, None)
            psm_att_cm.__exit__(None, None, None)
            psm_p3b_cm = tc.tile_pool(name="psm3b", bufs=1, space="PSUM")
            psm_p3b = psm_p3b_cm.__enter__()
            phase3(0, 2, psm_p3b)
            psm_p3b_cm.__exit__(None, None, None)

    nc.finalize()
    return nc


def kernel(x, Wq, bq, Wk, bk, Wv, bv, Wo, bo):
    if "nc" not in _CACHE:
        _CACHE["nc"] = _build()
    nc = _CACHE["nc"]

    x = np.asarray(x, dtype=np.float32)
    # permute Wo rows from global head-dim order (192g + 64h + d) to the
    # head-major gathered layout (256h + 64g + d) used by phase 3
    perm = np.empty(C, dtype=np.int64)
    for h_l in range(HPC):
        for g in range(4):
            perm[256 * h_l + 64 * g : 256 * h_l + 64 * g + 64] = np.arange(
                MPC * g + D * h_l, MPC * g + D * h_l + D
            )
    wo_send = np.ascontiguousarray(np.asarray(Wo, np.float32)[perm, :])
    in_maps = []
    for c in range(NCORES):
        b, g = c // 4, c % 4
        sl = slice(MPC * g, MPC * (g + 1))
        in_maps.append({
            "x": np.ascontiguousarray(x[b].T),
            "wq": np.ascontiguousarray(np.asarray(Wq, np.float32)[:, sl]),
            "wk": np.ascontiguousarray(np.asarray(Wk, np.float32)[:, sl]),
            "wv": np.ascontiguousarray(np.asarray(Wv, np.float32)[:, sl]),
            "bq": np.ascontiguousarray(np.asarray(bq, np.float32)[sl]),
            "bk": np.ascontiguousarray(np.asarray(bk, np.float32)[sl]),
            "bv": np.ascontiguousarray(np.asarray(bv, np.float32)[sl]),
            "wo": wo_send,
            "bo": np.ascontiguousarray(np.asarray(bo, np.float32)),
        })

    res = run_bass_kernel_spmd(nc, in_maps, core_ids=list(range(NCORES)))
    out = np.empty((2, T, C), dtype=np.float32)
    for j in range(NCORES):
        r = res.results[j]["out"]
        out[0, 512 * j : 512 * (j + 1), :] = r[0]
        out[1, 512 * j : 512 * (j + 1), :] = r[1]
    return out
